# revision 23
# baseline (speedup 1.0000x reference)
"""Trainium2 Bass kernel for nn_Attention_61443802137307.

Multi-head attention block:
    x_topo = x + topo_all_fea (if is_end)
    kv = x_topo @ kv_w.T ; q = x @ q_w.T (scale hd^-0.5 folded into q_w)
    attn = softmax(q k^T); out = (attn @ v) @ proj_w.T + proj_b

Sharding: data-parallel over batch (dim 0), 32 batches per core x 8 cores.

Per-core design (bf16 activations/weights, feature-major):
  - groups of 8 batches (1152 tokens = 9 full 128-token tiles)
  - q/k projections feature-major into 384-col psum chunks -> bf16 q_fm/k_fm
    (fc-major order so head pair pr is ready after part pr)
  - v token-major per batch ([128,512] psum) + one packed strided matmul
    for all 8 batches' 16-token tails; ones column for softmax sums
  - scoresT per (batch, head-pair): kA [128,2,144] psum; k-tails for 8
    batches packed into three [128,2,144] psum tiles at bases 0/32/64
  - exp on ACT; av contracts tokens, psum row 64 = softmax denominator
  - denominators: per-pr DMA gather, per-half-group DVE reciprocal,
    gpsimd partition_broadcast (Pool), bf16 DVE multiply -> attn_c
  - proj: 9 full [128,512] tiles per group, bias add split ACT/DVE
  - 2-stage pipeline: slot g runs A(g)+B(g) interleaved (B(pr) starts as
    soon as its q/k chunks + vA tiles exist) | C(g-1) = proj of previous
    group. Within B, score matmuls run one batch ahead of av (exp hiding).
"""
import numpy as np
from ml_dtypes import bfloat16

import concourse.bass as bass
import concourse.tile as tile
import concourse.mybir as mybir
from concourse import bacc
from concourse.bass_utils import run_bass_kernel_spmd
from contextlib import ExitStack

F32 = mybir.dt.float32
BF16 = mybir.dt.bfloat16
AF = mybir.ActivationFunctionType

B, N, D = 256, 144, 512
H, HD = 8, 64
SCALE = HD ** -0.5
N_CORES = 8
BPC = B // N_CORES          # 32 batches per core
TOK = BPC * N               # 4608 tokens per core
GB = 8                      # batches per group
NG = BPC // GB              # 4 groups
GTOK = GB * N               # 1152 tokens per group (= 9 * 128)
NTT = GTOK // 128           # 9 full token tiles per group
QKCH = 384                  # q/k projection psum chunk
NCH = GTOK // QKCH          # 3 chunks

_CACHE = {}


def build():
    nc = bacc.Bacc("TRN2", target_bir_lowering=False, debug=False,
                   num_devices=N_CORES)

    xT = nc.dram_tensor("xT", [4, 128, TOK], BF16, kind="ExternalInput").ap()
    xkT = nc.dram_tensor("xkT", [4, 128, TOK], BF16, kind="ExternalInput").ap()
    kv_wT = nc.dram_tensor("kv_wT", [4, 128, 2 * D], BF16,
                           kind="ExternalInput").ap()
    q_wT = nc.dram_tensor("q_wT", [4, 128, D], BF16, kind="ExternalInput").ap()
    p_wT = nc.dram_tensor("p_wT", [4, 128, D], BF16, kind="ExternalInput").ap()
    p_b = nc.dram_tensor("p_b", [D], F32, kind="ExternalInput").ap()
    out = nc.dram_tensor("out", [TOK, D], F32, kind="ExternalOutput").ap()

    with tile.TileContext(nc) as tc, ExitStack() as ctx:
        singles = ctx.enter_context(tc.tile_pool(name="singles", bufs=1))
        xpool = ctx.enter_context(tc.tile_pool(name="xpool", bufs=2))
        qkpool = ctx.enter_context(tc.tile_pool(name="qkpool", bufs=2))
        eApool = ctx.enter_context(tc.tile_pool(name="eApool", bufs=3))
        eBpool = ctx.enter_context(tc.tile_pool(name="eBpool", bufs=6))
        aupool = ctx.enter_context(tc.tile_pool(name="aupool", bufs=2))
        spool = ctx.enter_context(tc.tile_pool(name="spool", bufs=2))
        bcpool = ctx.enter_context(tc.tile_pool(name="bcpool", bufs=3))
        acpool = ctx.enter_context(tc.tile_pool(name="acpool", bufs=2))
        opool = ctx.enter_context(tc.tile_pool(name="opool", bufs=3))
        # 8 psum banks: pj 2 (q/k/v chunks + proj) + ms 6 (score/tail/av)
        ps_pj = ctx.enter_context(tc.tile_pool(name="ps_pj", bufs=2,
                                               space="PSUM"))
        ps_ms = ctx.enter_context(tc.tile_pool(name="ps_ms", bufs=6,
                                               space="PSUM"))

        # --- persistent weights ---
        kv_w_sb = singles.tile([128, 4, 2 * D], BF16)
        q_w_sb = singles.tile([128, 4, D], BF16)
        p_w_sb = singles.tile([128, 4, D], BF16)
        bias_bc = singles.tile([128, D], F32)

        # persistent v tiles (2 pipeline slots); ones columns memset once
        vA = [[singles.tile([128, H, 65], BF16, name=f"vA{s}_{b}")
               for b in range(GB)] for s in range(2)]
        vB = [[singles.tile([128, H, 65], BF16, name=f"vB{s}_{t}")
               for t in range(3)] for s in range(2)]

        def alloc_group_tiles(g):
            xg = xpool.tile([128, 4, GTOK], BF16, tag="xg", name=f"xg{g}")
            xkg = xpool.tile([128, 4, GTOK], BF16, tag="xkg", name=f"xkg{g}")
            q_fm = qkpool.tile([128, 4, GTOK], BF16, tag="qfm", name=f"qf{g}")
            k_fm = qkpool.tile([128, 4, GTOK], BF16, tag="kfm", name=f"kf{g}")
            return (xg, xkg, q_fm, k_fm)

        def a_load(g):
            g0 = g * GTOK
            st = alloc_group_tiles(g)
            xg, xkg = st[0], st[1]
            for kc in range(4):
                nc.sync.dma_start(xg[:, kc, :], xT[kc, :, g0:g0 + GTOK])
                nc.sync.dma_start(xkg[:, kc, :], xkT[kc, :, g0:g0 + GTOK])
            return st

        # prologue: interleave weight and group-0 activation DMAs so the
        # first q-projection matmuls start as early as possible
        st0 = alloc_group_tiles(0)
        for kc in range(4):
            nc.sync.dma_start(q_w_sb[:, kc, :], q_wT[kc])
            nc.sync.dma_start(st0[0][:, kc, :], xT[kc, :, 0:GTOK])
        for kc in range(4):
            nc.sync.dma_start(kv_w_sb[:, kc, :], kv_wT[kc])
            nc.sync.dma_start(st0[1][:, kc, :], xkT[kc, :, 0:GTOK])
        for kc in range(4):
            nc.sync.dma_start(p_w_sb[:, kc, :], p_wT[kc])
        bias_src = bass.AP(tensor=p_b.tensor, offset=0, ap=[[0, 128], [1, D]])
        nc.gpsimd.dma_start(out=bias_bc[:], in_=bias_src)
        for s in range(2):
            for b in range(GB):
                nc.gpsimd.memset(vA[s][b][:, :, 64:65], 1.0)
            for t in range(3):
                nc.gpsimd.memset(vB[s][t][:, :, 64:65], 1.0)

        def a_part(g, i, st):
            """i in 0..3: q/k chunks for fc=i, 2 vA batches, tail at i=1."""
            xg, xkg, q_fm, k_fm = st
            s = g % 2
            if i == 1:
                # packed v-tail: all 8 batches' tokens 128:144 in one matmul
                tp = ps_pj.tile([128, D], F32, tag="pj", name=f"pjt{g}")
                xkt = xkg[:].rearrange("p f (b c) -> p f b c", c=N)
                for kc in range(4):
                    nc.tensor.matmul(
                        tp[:], xkt[:, kc, :, 128:144],
                        kv_w_sb[:, kc, D:2 * D],
                        start=(kc == 0), stop=(kc == 3))
                tpv = tp[:].rearrange("p (h d) -> p h d", h=H)
                for b in range(GB):
                    dst = vB[s][b // 3][32 * (b % 3):32 * (b % 3) + 16, :,
                                        0:64]
                    if b % 2 == 0:
                        nc.scalar.copy(dst, tpv[16 * b:16 * b + 16])
                    else:
                        nc.vector.tensor_copy(dst, tpv[16 * b:16 * b + 16])
            fc = i
            for isq in range(2):
                src = xg if isq == 0 else xkg
                w_sb = q_w_sb if isq == 0 else kv_w_sb
                dst = q_fm if isq == 0 else k_fm
                for ch in range(NCH):
                    p = ps_pj.tile([128, QKCH], F32, tag="pj",
                                   name=f"pj{g}_{i}_{isq}{ch}")
                    for kc in range(4):
                        nc.tensor.matmul(
                            p[:],
                            w_sb[:, kc, 128 * fc:128 * fc + 128],
                            src[:, kc, QKCH * ch:QKCH * ch + QKCH],
                            start=(kc == 0), stop=(kc == 3))
                    nc.vector.tensor_copy(
                        dst[:, fc, QKCH * ch:QKCH * ch + QKCH], p[:])
            for b in (2 * i, 2 * i + 1):
                off = N * b
                p = ps_pj.tile([128, D], F32, tag="pj", name=f"pjv{g}_{b}")
                for kc in range(4):
                    nc.tensor.matmul(
                        p[:], xkg[:, kc, off:off + 128],
                        kv_w_sb[:, kc, D:2 * D],
                        start=(kc == 0), stop=(kc == 3))
                nc.scalar.copy(vA[s][b][:, :, 0:64],
                               p[:].rearrange("p (h d) -> p h d", h=H))

        def b_start(g):
            au = aupool.tile([65, H, GTOK], BF16, tag="au", name=f"au{g}")
            sums = spool.tile([8, GTOK], BF16, tag="sums", name=f"sm{g}")
            recip_bf = spool.tile([8, GTOK], BF16, tag="recipb",
                                  name=f"rb{g}")
            attn_c = acpool.tile([128, 4, GTOK], BF16, tag="ac", name=f"ac{g}")
            return (au, sums, recip_bf, attn_c)

        def b_sc(g, pr, b, q_fm, k_fm):
            """scoresT kA matmuls + exp for one batch."""
            off = N * b
            sc = ps_ms.tile([128, 2, N], F32, tag="ms", name=f"sc{g}_{pr}{b}")
            for j in range(2):
                r0 = 64 * j
                nc.tensor.matmul(
                    sc[:, j, :],
                    k_fm[r0:r0 + 64, pr, off:off + 128],
                    q_fm[r0:r0 + 64, pr, off:off + N],
                    start=True, stop=True)
            e1A = eApool.tile([128, 2, N], BF16, tag="eA", name=f"eA{g}_{b}")
            nc.scalar.activation(e1A[:], sc[:], AF.Exp)
            return e1A

        def b_av(g, pr, b, e1A, e1B, au):
            s = g % 2
            t, pb = b // 3, 32 * (b % 3)
            off = N * b
            av = ps_ms.tile([128, 2, 256], F32, tag="ms",
                            name=f"av{g}_{b}_{pr}")
            for j in range(2):
                h = 2 * pr + j
                nc.tensor.matmul(av[0:65, j, 0:N], vA[s][b][:, h, :],
                                 e1A[:, j, :], start=True, stop=False)
                nc.tensor.matmul(av[0:65, j, 0:N],
                                 vB[s][t][pb:pb + 16, h, :],
                                 e1B[t][pb:pb + 16, j, :],
                                 start=False, stop=True)
            dst = au[:, 2 * pr:2 * pr + 2, off:off + N]
            if b % 2 == 0:
                nc.scalar.copy(dst, av[0:65, :, 0:N])
            else:
                nc.vector.tensor_copy(dst, av[0:65, :, 0:N])

        def b_part(g, i, st, bst, stash):
            """i in 2..9: idx = i-2 -> (pr, half); 4 batches per part.

            Score matmuls run one batch ahead of av so exp latency hides
            behind the next batch's PE work.
            """
            _, _, q_fm, k_fm = st
            au, sums, recip_bf, attn_c = bst
            idx = i - 2
            pr, half = idx // 2, idx % 2
            if half == 0:
                # packed k-tail scores for all 8 batches of this head pair
                tl = [ps_ms.tile([128, 2, N], F32, tag="ms",
                                 name=f"tl{g}_{pr}_{t}") for t in range(3)]
                for b in range(GB):
                    t, pb = b // 3, 32 * (b % 3)
                    off = N * b
                    for j in range(2):
                        r0 = 64 * j
                        nc.tensor.matmul(
                            tl[t][pb:pb + 16, j, :],
                            k_fm[r0:r0 + 64, pr, off + 128:off + 144],
                            q_fm[r0:r0 + 64, pr, off:off + N],
                            start=True, stop=True)
                e1B = [eBpool.tile([128, 2, N], BF16, tag="eB",
                                   name=f"eB{g}_{pr}_{t}") for t in range(3)]
                for t in range(3):
                    nc.scalar.activation(e1B[t][:], tl[t][:], AF.Exp)
                stash["eB"] = e1B
                stash["pend"] = None
            e1B = stash["eB"]
            for b in range(4 * half, 4 * half + 4):
                e1A = b_sc(g, pr, b, q_fm, k_fm)
                if stash["pend"] is not None:
                    pb_, e1A_ = stash["pend"]
                    b_av(g, pr, pb_, e1A_, e1B, au)
                stash["pend"] = (b, e1A)
            if half == 1:
                pb_, e1A_ = stash["pend"]
                b_av(g, pr, pb_, e1A_, e1B, au)
                stash["pend"] = None
                h0 = 2 * pr
                nc.sync.dma_start(sums[h0:h0 + 2, :],
                                  au[64:65, h0:h0 + 2, :])
                if pr % 2 == 1:
                    # normalize the finished half-group (heads 2pr-2..2pr+1).
                    # bf16 reciprocal: downstream math is bf16 anyway.
                    r0 = 2 * pr - 2
                    with nc.allow_low_precision("softmax recip used in bf16"):
                        nc.vector.reciprocal(recip_bf[r0:r0 + 4, :],
                                             sums[r0:r0 + 4, :])
                    HT = GTOK // 2
                    for h in range(r0, r0 + 4):
                        for cj in range(2):
                            c0 = HT * cj
                            bc = bcpool.tile([64, HT], BF16, tag="bc",
                                             name=f"bc{g}_{h}_{cj}")
                            nc.gpsimd.partition_broadcast(
                                bc[:], recip_bf[h:h + 1, c0:c0 + HT])
                            nc.vector.tensor_mul(
                                attn_c[64 * (h % 2):64 * (h % 2) + 64,
                                       h // 2, c0:c0 + HT],
                                au[0:64, h, c0:c0 + HT], bc[:])

        # proj tile indices per part i
        proj_map = {3: (0,), 4: (1,), 5: (2,), 6: (3,), 7: (4,),
                    8: (5, 6), 9: (7, 8)}

        def c_part(g, i, bst):
            attn_c = bst[3]
            g0 = g * GTOK
            for t in proj_map.get(i, ()):
                p = ps_pj.tile([128, D], F32, tag="pj", name=f"pp{g}_{t}")
                for fc in range(4):
                    nc.tensor.matmul(
                        p[:], attn_c[:, fc, 128 * t:128 * t + 128],
                        p_w_sb[:, fc, :],
                        start=(fc == 0), stop=(fc == 3))
                o_sb = opool.tile([128, D], F32, tag="osb", name=f"o{g}_{t}")
                nc.vector.tensor_add(o_sb[:], p[:], bias_bc[:])
                nc.sync.dma_start(
                    out[g0 + 128 * t:g0 + 128 * t + 128, :], o_sb[:])

        # 2-stage pipeline: slot g: A(g)+B(g) | C(g-1)
        state = {}
        bst = {}
        stash = {}
        state[0] = st0
        for g in range(NG + 1):
            if 0 < g + 1 < NG:
                state[g + 1] = a_load(g + 1)
            if g < NG:
                bst[g] = b_start(g)
                stash[g] = {"eB": None, "pend": None}
            for i in range(10):
                if g < NG and i < 4:
                    a_part(g, i, state[g])
                if 0 <= g - 1 < NG:
                    c_part(g - 1, i, bst[g - 1])
                if g < NG and 2 <= i:
                    b_part(g, i, state[g], bst[g], stash[g])
            state.pop(g, None)
            stash.pop(g, None)
            bst.pop(g - 1, None)

    nc.compile()
    return nc


def _get_nc():
    if "nc" not in _CACHE:
        _CACHE["nc"] = build()
    return _CACHE["nc"]


def _fm_bf16(a):
    """[tok, D] f32 -> [4, 128, tok] feature-major bf16 chunks."""
    t = np.ascontiguousarray(a.reshape(-1, D).T.astype(bfloat16))
    return t.reshape(4, 128, -1)


def kernel(x, topo_all_fea, kv_w, q_w, proj_w, proj_b, is_end):
    x = np.asarray(x, dtype=np.float32)
    topo = np.asarray(topo_all_fea, dtype=np.float32)
    kv_w = np.asarray(kv_w, dtype=np.float32)
    q_w = np.asarray(q_w, dtype=np.float32)
    proj_w = np.asarray(proj_w, dtype=np.float32)
    proj_b = np.asarray(proj_b, dtype=np.float32)
    end = bool(np.asarray(is_end).item()) if not isinstance(is_end, bool) \
        else is_end

    xk = x + topo if end else x

    kv_wT = np.ascontiguousarray(kv_w.T).astype(bfloat16).reshape(4, 128,
                                                                  2 * D)
    q_wT = np.ascontiguousarray(q_w.T * SCALE).astype(bfloat16).reshape(
        4, 128, D)
    p_wT = np.ascontiguousarray(proj_w.T).astype(bfloat16).reshape(4, 128, D)

    nc = _get_nc()
    in_maps = [
        {"xT": _fm_bf16(x[c * BPC:(c + 1) * BPC]),
         "xkT": _fm_bf16(xk[c * BPC:(c + 1) * BPC]),
         "kv_wT": kv_wT, "q_wT": q_wT, "p_wT": p_wT, "p_b": proj_b}
        for c in range(N_CORES)
    ]
    res = run_bass_kernel_spmd(nc, in_maps, core_ids=list(range(N_CORES)))
    outs = [res.results[c]["out"].reshape(BPC, N, D) for c in range(N_CORES)]
    return np.concatenate(outs, axis=0)


# revision 29
# speedup vs baseline: 1.0144x; 1.0144x over previous
"""Trainium2 Bass kernel for nn_Attention_61443802137307.

Multi-head attention block:
    x_topo = x + topo_all_fea (if is_end)
    kv = x_topo @ kv_w.T ; q = x @ q_w.T (scale hd^-0.5 folded into q_w)
    attn = softmax(q k^T); out = (attn @ v) @ proj_w.T + proj_b

Sharding: data-parallel over batch (dim 0), 32 batches per core x 8 cores.

Per-core design (bf16 activations/weights, feature-major):
  - groups of 8 batches (1152 tokens = 9 full 128-token tiles)
  - q/k projections feature-major into 384-col psum chunks -> bf16 q_fm/k_fm
    (fc-major order so head pair pr is ready after part pr)
  - v token-major per batch ([128,512] psum) + one packed strided matmul
    for all 8 batches' 16-token tails; ones column for softmax sums
  - scoresT per (batch, head-pair): kA [128,2,144] psum; k-tails for 8
    batches packed into three [128,2,144] psum tiles at bases 0/32/64
  - exp on ACT; av contracts tokens, psum row 64 = softmax denominator
  - denominators: per-pr DMA gather, per-half-group DVE reciprocal,
    gpsimd partition_broadcast (Pool), bf16 DVE multiply -> attn_c
  - proj: 9 full [128,512] tiles per group, bias add split ACT/DVE
  - 2-stage pipeline: slot g runs A(g)+B(g) interleaved (B(pr) starts as
    soon as its q/k chunks + vA tiles exist) | C(g-1) = proj of previous
    group. Within B, score matmuls run one batch ahead of av (exp hiding).
"""
import numpy as np
from ml_dtypes import bfloat16

import concourse.bass as bass
import concourse.tile as tile
import concourse.mybir as mybir
from concourse import bacc
from concourse.bass_utils import run_bass_kernel_spmd
from contextlib import ExitStack

F32 = mybir.dt.float32
BF16 = mybir.dt.bfloat16
AF = mybir.ActivationFunctionType

B, N, D = 256, 144, 512
H, HD = 8, 64
SCALE = HD ** -0.5
N_CORES = 8
BPC = B // N_CORES          # 32 batches per core
TOK = BPC * N               # 4608 tokens per core
GB = 8                      # batches per group
NG = BPC // GB              # 4 groups
GTOK = GB * N               # 1152 tokens per group (= 9 * 128)
NTT = GTOK // 128           # 9 full token tiles per group
QKCH = 384                  # q/k projection psum chunk
NCH = GTOK // QKCH          # 3 chunks

_CACHE = {}


def build():
    nc = bacc.Bacc("TRN2", target_bir_lowering=False, debug=False,
                   num_devices=N_CORES)

    xT = nc.dram_tensor("xT", [4, 128, TOK], BF16, kind="ExternalInput").ap()
    xkT = nc.dram_tensor("xkT", [4, 128, TOK], BF16, kind="ExternalInput").ap()
    kv_wT = nc.dram_tensor("kv_wT", [4, 128, 2 * D], BF16,
                           kind="ExternalInput").ap()
    q_wT = nc.dram_tensor("q_wT", [4, 128, D], BF16, kind="ExternalInput").ap()
    p_wT = nc.dram_tensor("p_wT", [4, 128, D], BF16, kind="ExternalInput").ap()
    p_b = nc.dram_tensor("p_b", [D], F32, kind="ExternalInput").ap()
    out = nc.dram_tensor("out", [TOK, D], F32, kind="ExternalOutput").ap()

    with tile.TileContext(nc) as tc, ExitStack() as ctx:
        singles = ctx.enter_context(tc.tile_pool(name="singles", bufs=1))
        xpool = ctx.enter_context(tc.tile_pool(name="xpool", bufs=2))
        qkpool = ctx.enter_context(tc.tile_pool(name="qkpool", bufs=2))
        eApool = ctx.enter_context(tc.tile_pool(name="eApool", bufs=3))
        eBpool = ctx.enter_context(tc.tile_pool(name="eBpool", bufs=6))
        aupool = ctx.enter_context(tc.tile_pool(name="aupool", bufs=2))
        spool = ctx.enter_context(tc.tile_pool(name="spool", bufs=2))
        bcpool = ctx.enter_context(tc.tile_pool(name="bcpool", bufs=3))
        acpool = ctx.enter_context(tc.tile_pool(name="acpool", bufs=2))
        opool = ctx.enter_context(tc.tile_pool(name="opool", bufs=3))
        # 8 psum banks: pj 2 (q/k/v chunks + proj) + ms 6 (score/tail/av)
        ps_pj = ctx.enter_context(tc.tile_pool(name="ps_pj", bufs=2,
                                               space="PSUM"))
        ps_ms = ctx.enter_context(tc.tile_pool(name="ps_ms", bufs=6,
                                               space="PSUM"))

        # --- persistent weights ---
        kv_w_sb = singles.tile([128, 4, 2 * D], BF16)
        q_w_sb = singles.tile([128, 4, D], BF16)
        p_w_sb = singles.tile([128, 4, D], BF16)
        bias_bc = singles.tile([128, D], F32)

        # persistent v tiles (2 pipeline slots); ones columns memset once
        vA = [[singles.tile([128, H, 65], BF16, name=f"vA{s}_{b}")
               for b in range(GB)] for s in range(2)]
        vB = [[singles.tile([128, H, 65], BF16, name=f"vB{s}_{t}")
               for t in range(3)] for s in range(2)]

        def alloc_group_tiles(g):
            xg = xpool.tile([128, 4, GTOK], BF16, tag="xg", name=f"xg{g}")
            xkg = xpool.tile([128, 4, GTOK], BF16, tag="xkg", name=f"xkg{g}")
            q_fm = qkpool.tile([128, 4, GTOK], BF16, tag="qfm", name=f"qf{g}")
            k_fm = qkpool.tile([128, 4, GTOK], BF16, tag="kfm", name=f"kf{g}")
            return (xg, xkg, q_fm, k_fm)

        def a_load(g):
            g0 = g * GTOK
            st = alloc_group_tiles(g)
            xg, xkg = st[0], st[1]
            for kc in range(4):
                nc.sync.dma_start(xg[:, kc, :], xT[kc, :, g0:g0 + GTOK])
                nc.sync.dma_start(xkg[:, kc, :], xkT[kc, :, g0:g0 + GTOK])
            return st

        # prologue: interleave weight and group-0 activation DMAs so the
        # first q-projection matmuls start as early as possible
        st0 = alloc_group_tiles(0)
        for kc in range(4):
            nc.sync.dma_start(q_w_sb[:, kc, :], q_wT[kc])
            nc.sync.dma_start(st0[0][:, kc, :], xT[kc, :, 0:GTOK])
        for kc in range(4):
            nc.sync.dma_start(kv_w_sb[:, kc, :], kv_wT[kc])
            nc.sync.dma_start(st0[1][:, kc, :], xkT[kc, :, 0:GTOK])
        for kc in range(4):
            nc.sync.dma_start(p_w_sb[:, kc, :], p_wT[kc])
        bias_src = bass.AP(tensor=p_b.tensor, offset=0, ap=[[0, 128], [1, D]])
        nc.gpsimd.dma_start(out=bias_bc[:], in_=bias_src)
        for s in range(2):
            for b in range(GB):
                nc.gpsimd.memset(vA[s][b][:, :, 64:65], 1.0)
            for t in range(3):
                nc.gpsimd.memset(vB[s][t][:, :, 64:65], 1.0)

        def a_part(g, i, st):
            """i in 0..3: q/k chunks for fc=i, 2 vA batches, tail at i=1."""
            xg, xkg, q_fm, k_fm = st
            s = g % 2
            if i == 1:
                # packed v-tail: all 8 batches' tokens 128:144 in one matmul
                tp = ps_pj.tile([128, D], F32, tag="pj", name=f"pjt{g}")
                xkt = xkg[:].rearrange("p f (b c) -> p f b c", c=N)
                for kc in range(4):
                    nc.tensor.matmul(
                        tp[:], xkt[:, kc, :, 128:144],
                        kv_w_sb[:, kc, D:2 * D],
                        start=(kc == 0), stop=(kc == 3))
                tpv = tp[:].rearrange("p (h d) -> p h d", h=H)
                for b in range(GB):
                    dst = vB[s][b // 3][32 * (b % 3):32 * (b % 3) + 16, :,
                                        0:64]
                    nc.scalar.copy(dst, tpv[16 * b:16 * b + 16])
            fc = i
            for isq in range(2):
                src = xg if isq == 0 else xkg
                w_sb = q_w_sb if isq == 0 else kv_w_sb
                dst = q_fm if isq == 0 else k_fm
                for ch in range(NCH):
                    p = ps_pj.tile([128, QKCH], F32, tag="pj",
                                   name=f"pj{g}_{i}_{isq}{ch}")
                    for kc in range(4):
                        nc.tensor.matmul(
                            p[:],
                            w_sb[:, kc, 128 * fc:128 * fc + 128],
                            src[:, kc, QKCH * ch:QKCH * ch + QKCH],
                            start=(kc == 0), stop=(kc == 3))
                    cdst = dst[:, fc, QKCH * ch:QKCH * ch + QKCH]
                    if (ch + isq) % 2 == 0:
                        nc.vector.tensor_copy(cdst, p[:])
                    else:
                        nc.scalar.copy(cdst, p[:])
            for b in (2 * i, 2 * i + 1):
                off = N * b
                p = ps_pj.tile([128, D], F32, tag="pj", name=f"pjv{g}_{b}")
                for kc in range(4):
                    nc.tensor.matmul(
                        p[:], xkg[:, kc, off:off + 128],
                        kv_w_sb[:, kc, D:2 * D],
                        start=(kc == 0), stop=(kc == 3))
                nc.scalar.copy(vA[s][b][:, :, 0:64],
                               p[:].rearrange("p (h d) -> p h d", h=H))

        def b_start(g):
            au = aupool.tile([65, H, GTOK], BF16, tag="au", name=f"au{g}")
            sums = spool.tile([8, GTOK], BF16, tag="sums", name=f"sm{g}")
            recip_bf = spool.tile([8, GTOK], BF16, tag="recipb",
                                  name=f"rb{g}")
            attn_c = acpool.tile([128, 4, GTOK], BF16, tag="ac", name=f"ac{g}")
            return (au, sums, recip_bf, attn_c)

        def b_sc(g, pr, b, q_fm, k_fm):
            """scoresT kA matmuls + exp for one batch."""
            off = N * b
            sc = ps_ms.tile([128, 2, N], F32, tag="ms", name=f"sc{g}_{pr}{b}")
            for j in range(2):
                r0 = 64 * j
                nc.tensor.matmul(
                    sc[:, j, :],
                    k_fm[r0:r0 + 64, pr, off:off + 128],
                    q_fm[r0:r0 + 64, pr, off:off + N],
                    start=True, stop=True)
            e1A = eApool.tile([128, 2, N], BF16, tag="eA", name=f"eA{g}_{b}")
            nc.scalar.activation(e1A[:], sc[:], AF.Exp)
            return e1A

        def b_av(g, pr, b, e1A, e1B, au):
            s = g % 2
            t, pb = b // 3, 32 * (b % 3)
            off = N * b
            av = ps_ms.tile([128, 2, 256], F32, tag="ms",
                            name=f"av{g}_{b}_{pr}")
            for j in range(2):
                h = 2 * pr + j
                nc.tensor.matmul(av[0:65, j, 0:N], vA[s][b][:, h, :],
                                 e1A[:, j, :], start=True, stop=False)
                nc.tensor.matmul(av[0:65, j, 0:N],
                                 vB[s][t][pb:pb + 16, h, :],
                                 e1B[t][pb:pb + 16, j, :],
                                 start=False, stop=True)
            dst = au[:, 2 * pr:2 * pr + 2, off:off + N]
            nc.vector.tensor_copy(dst, av[0:65, :, 0:N])

        def b_part(g, i, st, bst, stash):
            """i in 2..9: idx = i-2 -> (pr, half); 4 batches per part.

            Score matmuls run one batch ahead of av so exp latency hides
            behind the next batch's PE work.
            """
            _, _, q_fm, k_fm = st
            au, sums, recip_bf, attn_c = bst
            idx = i - 2
            pr, half = idx // 2, idx % 2
            if half == 0:
                # packed k-tail scores for all 8 batches of this head pair
                tl = [ps_ms.tile([128, 2, N], F32, tag="ms",
                                 name=f"tl{g}_{pr}_{t}") for t in range(3)]
                for b in range(GB):
                    t, pb = b // 3, 32 * (b % 3)
                    off = N * b
                    for j in range(2):
                        r0 = 64 * j
                        nc.tensor.matmul(
                            tl[t][pb:pb + 16, j, :],
                            k_fm[r0:r0 + 64, pr, off + 128:off + 144],
                            q_fm[r0:r0 + 64, pr, off:off + N],
                            start=True, stop=True)
                e1B = [eBpool.tile([128, 2, N], BF16, tag="eB",
                                   name=f"eB{g}_{pr}_{t}") for t in range(3)]
                for t in range(3):
                    nc.scalar.activation(e1B[t][:], tl[t][:], AF.Exp)
                stash["eB"] = e1B
            e1B = stash["eB"]
            pend = stash["pend"]
            for b in range(4 * half, 4 * half + 4):
                e1A = b_sc(g, pr, b, q_fm, k_fm)
                pend.append((b, e1A))
                if len(pend) > 2:
                    pb_, e1A_ = pend.pop(0)
                    b_av(g, pr, pb_, e1A_, e1B, au)
            if half == 1:
                while pend:
                    pb_, e1A_ = pend.pop(0)
                    b_av(g, pr, pb_, e1A_, e1B, au)
                h0 = 2 * pr
                nc.sync.dma_start(sums[h0:h0 + 2, :],
                                  au[64:65, h0:h0 + 2, :])
                last_g = (g == NG - 1)
                if last_g or pr % 2 == 1:
                    # normalize finished head pairs.  bf16 reciprocal:
                    # downstream math is bf16 anyway.  For the last group
                    # normalize per pair to shorten the drain chain.
                    r0 = 2 * pr if last_g else 2 * pr - 2
                    nh = 2 if last_g else 4
                    with nc.allow_low_precision("softmax recip used in bf16"):
                        nc.vector.reciprocal(recip_bf[r0:r0 + nh, :],
                                             sums[r0:r0 + nh, :])
                    HT = GTOK // 2
                    for cj in range(2):
                        c0 = HT * cj
                        for h in range(r0, r0 + nh):
                            bc = bcpool.tile([64, HT], BF16, tag="bc",
                                             name=f"bc{g}_{h}_{cj}")
                            nc.gpsimd.partition_broadcast(
                                bc[:], recip_bf[h:h + 1, c0:c0 + HT])
                            nc.vector.tensor_mul(
                                attn_c[64 * (h % 2):64 * (h % 2) + 64,
                                       h // 2, c0:c0 + HT],
                                au[0:64, h, c0:c0 + HT], bc[:])

        # proj tile indices per part i
        proj_map = {3: (0,), 4: (1,), 5: (2,), 6: (3,), 7: (4,),
                    8: (5, 6), 9: (7, 8)}

        def c_part(g, i, bst):
            attn_c = bst[3]
            g0 = g * GTOK
            for t in proj_map.get(i, ()):
                p = ps_pj.tile([128, D], F32, tag="pj", name=f"pp{g}_{t}")
                for fc in range(4):
                    nc.tensor.matmul(
                        p[:], attn_c[:, fc, 128 * t:128 * t + 128],
                        p_w_sb[:, fc, :],
                        start=(fc == 0), stop=(fc == 3))
                o_sb = opool.tile([128, D], F32, tag="osb", name=f"o{g}_{t}")
                nc.vector.tensor_add(o_sb[:], p[:], bias_bc[:])
                nc.sync.dma_start(
                    out[g0 + 128 * t:g0 + 128 * t + 128, :], o_sb[:])

        # 2-stage pipeline: slot g: A(g)+B(g) | C(g-1)
        state = {}
        bst = {}
        stash = {}
        state[0] = st0
        for g in range(NG + 1):
            if 0 < g + 1 < NG:
                state[g + 1] = a_load(g + 1)
            if g < NG:
                bst[g] = b_start(g)
                stash[g] = {"eB": None, "pend": []}
            for i in range(10):
                if g < NG and i < 4:
                    a_part(g, i, state[g])
                if 0 <= g - 1 < NG:
                    c_part(g - 1, i, bst[g - 1])
                if g < NG and 2 <= i:
                    b_part(g, i, state[g], bst[g], stash[g])
            state.pop(g, None)
            stash.pop(g, None)
            bst.pop(g - 1, None)

    nc.compile()
    return nc


def _get_nc():
    if "nc" not in _CACHE:
        _CACHE["nc"] = build()
    return _CACHE["nc"]


def _fm_bf16(a):
    """[tok, D] f32 -> [4, 128, tok] feature-major bf16 chunks."""
    t = np.ascontiguousarray(a.reshape(-1, D).T.astype(bfloat16))
    return t.reshape(4, 128, -1)


def kernel(x, topo_all_fea, kv_w, q_w, proj_w, proj_b, is_end):
    x = np.asarray(x, dtype=np.float32)
    topo = np.asarray(topo_all_fea, dtype=np.float32)
    kv_w = np.asarray(kv_w, dtype=np.float32)
    q_w = np.asarray(q_w, dtype=np.float32)
    proj_w = np.asarray(proj_w, dtype=np.float32)
    proj_b = np.asarray(proj_b, dtype=np.float32)
    end = bool(np.asarray(is_end).item()) if not isinstance(is_end, bool) \
        else is_end

    xk = x + topo if end else x

    kv_wT = np.ascontiguousarray(kv_w.T).astype(bfloat16).reshape(4, 128,
                                                                  2 * D)
    q_wT = np.ascontiguousarray(q_w.T * SCALE).astype(bfloat16).reshape(
        4, 128, D)
    p_wT = np.ascontiguousarray(proj_w.T).astype(bfloat16).reshape(4, 128, D)

    nc = _get_nc()
    in_maps = [
        {"xT": _fm_bf16(x[c * BPC:(c + 1) * BPC]),
         "xkT": _fm_bf16(xk[c * BPC:(c + 1) * BPC]),
         "kv_wT": kv_wT, "q_wT": q_wT, "p_wT": p_wT, "p_b": proj_b}
        for c in range(N_CORES)
    ]
    res = run_bass_kernel_spmd(nc, in_maps, core_ids=list(range(N_CORES)))
    outs = [res.results[c]["out"].reshape(BPC, N, D) for c in range(N_CORES)]
    return np.concatenate(outs, axis=0)


# revision 33
# speedup vs baseline: 1.0274x; 1.0128x over previous
"""Trainium2 Bass kernel for nn_Attention_61443802137307.

Multi-head attention block:
    x_topo = x + topo_all_fea (if is_end)
    kv = x_topo @ kv_w.T ; q = x @ q_w.T (scale hd^-0.5 folded into q_w)
    attn = softmax(q k^T); out = (attn @ v) @ proj_w.T + proj_b

Sharding: data-parallel over batch (dim 0), 32 batches per core x 8 cores.

Per-core design (bf16 activations/weights, feature-major):
  - groups of 8 batches (1152 tokens = 9 full 128-token tiles)
  - q/k projections feature-major into 384-col psum chunks -> bf16 q_fm/k_fm
    (fc-major order so head pair pr is ready after part pr)
  - v token-major per batch ([128,512] psum) + one packed strided matmul
    for all 8 batches' 16-token tails; ones column for softmax sums
  - scoresT per (batch, head-pair): kA [128,2,144] psum; k-tails for 8
    batches packed into three [128,2,144] psum tiles at bases 0/32/64
  - exp on ACT; av contracts tokens, psum row 64 = softmax denominator
  - denominators: per-pr DMA gather, per-half-group DVE reciprocal,
    gpsimd partition_broadcast (Pool), bf16 DVE multiply -> attn_c
  - proj: 9 full [128,512] tiles per group, bias add split ACT/DVE
  - 2-stage pipeline: slot g runs A(g)+B(g) interleaved (B(pr) starts as
    soon as its q/k chunks + vA tiles exist) | C(g-1) = proj of previous
    group. Within B, score matmuls run one batch ahead of av (exp hiding).
"""
import numpy as np
from ml_dtypes import bfloat16

import concourse.bass as bass
import concourse.tile as tile
import concourse.mybir as mybir
from concourse import bacc
from concourse.bass_utils import run_bass_kernel_spmd
from contextlib import ExitStack

F32 = mybir.dt.float32
BF16 = mybir.dt.bfloat16
AF = mybir.ActivationFunctionType

B, N, D = 256, 144, 512
H, HD = 8, 64
SCALE = HD ** -0.5
N_CORES = 8
BPC = B // N_CORES          # 32 batches per core
TOK = BPC * N               # 4608 tokens per core
GB = 8                      # batches per group
NG = BPC // GB              # 4 groups
GTOK = GB * N               # 1152 tokens per group (= 9 * 128)
NTT = GTOK // 128           # 9 full token tiles per group
QKCH = 384                  # q/k projection psum chunk
NCH = GTOK // QKCH          # 3 chunks

_CACHE = {}


def build():
    nc = bacc.Bacc("TRN2", target_bir_lowering=False, debug=False,
                   num_devices=N_CORES)

    xT = nc.dram_tensor("xT", [4, 128, TOK], BF16, kind="ExternalInput").ap()
    xkT = nc.dram_tensor("xkT", [4, 128, TOK], BF16, kind="ExternalInput").ap()
    kv_wT = nc.dram_tensor("kv_wT", [4, 128, 2 * D], BF16,
                           kind="ExternalInput").ap()
    q_wT = nc.dram_tensor("q_wT", [4, 128, D], BF16, kind="ExternalInput").ap()
    p_wT = nc.dram_tensor("p_wT", [4, 128, D], BF16, kind="ExternalInput").ap()
    p_b = nc.dram_tensor("p_b", [D], F32, kind="ExternalInput").ap()
    out = nc.dram_tensor("out", [TOK, D], F32, kind="ExternalOutput").ap()

    with tile.TileContext(nc) as tc, ExitStack() as ctx:
        singles = ctx.enter_context(tc.tile_pool(name="singles", bufs=1))
        xpool = ctx.enter_context(tc.tile_pool(name="xpool", bufs=2))
        qkpool = ctx.enter_context(tc.tile_pool(name="qkpool", bufs=2))
        eApool = ctx.enter_context(tc.tile_pool(name="eApool", bufs=3))
        eBpool = ctx.enter_context(tc.tile_pool(name="eBpool", bufs=6))
        aupool = ctx.enter_context(tc.tile_pool(name="aupool", bufs=2))
        spool = ctx.enter_context(tc.tile_pool(name="spool", bufs=2))
        bcpool = ctx.enter_context(tc.tile_pool(name="bcpool", bufs=3))
        acpool = ctx.enter_context(tc.tile_pool(name="acpool", bufs=2))
        opool = ctx.enter_context(tc.tile_pool(name="opool", bufs=3))
        # 8 psum banks: pj 2 (q/k/v chunks + proj) + ms 6 (score/tail/av)
        ps_pj = ctx.enter_context(tc.tile_pool(name="ps_pj", bufs=2,
                                               space="PSUM"))
        ps_ms = ctx.enter_context(tc.tile_pool(name="ps_ms", bufs=6,
                                               space="PSUM"))

        # --- persistent weights ---
        kv_w_sb = singles.tile([128, 4, 2 * D], BF16)
        q_w_sb = singles.tile([128, 4, D], BF16)
        p_w_sb = singles.tile([128, 4, D], BF16)
        bias_bc = singles.tile([128, D], F32)

        # persistent v tiles (2 pipeline slots); ones columns memset once
        vA = [[singles.tile([128, H, 65], BF16, name=f"vA{s}_{b}")
               for b in range(GB)] for s in range(2)]
        vB = [[singles.tile([128, H, 65], BF16, name=f"vB{s}_{t}")
               for t in range(3)] for s in range(2)]

        def alloc_group_tiles(g):
            xg = xpool.tile([128, 4, GTOK], BF16, tag="xg", name=f"xg{g}")
            xkg = xpool.tile([128, 4, GTOK], BF16, tag="xkg", name=f"xkg{g}")
            q_fm = qkpool.tile([128, 4, GTOK], BF16, tag="qfm", name=f"qf{g}")
            k_fm = qkpool.tile([128, 4, GTOK], BF16, tag="kfm", name=f"kf{g}")
            return (xg, xkg, q_fm, k_fm)

        def a_load(g):
            g0 = g * GTOK
            st = alloc_group_tiles(g)
            xg, xkg = st[0], st[1]
            for kc in range(4):
                nc.sync.dma_start(xg[:, kc, :], xT[kc, :, g0:g0 + GTOK])
                nc.sync.dma_start(xkg[:, kc, :], xkT[kc, :, g0:g0 + GTOK])
            return st

        # prologue: interleave weight and group-0 activation DMAs so the
        # first q-projection matmuls start as early as possible
        st0 = alloc_group_tiles(0)
        for kc in range(4):
            nc.sync.dma_start(q_w_sb[:, kc, :], q_wT[kc])
            nc.sync.dma_start(st0[0][:, kc, :], xT[kc, :, 0:GTOK])
        for kc in range(4):
            nc.sync.dma_start(kv_w_sb[:, kc, :], kv_wT[kc])
            nc.sync.dma_start(st0[1][:, kc, :], xkT[kc, :, 0:GTOK])
        for kc in range(4):
            nc.sync.dma_start(p_w_sb[:, kc, :], p_wT[kc])
        bias_src = bass.AP(tensor=p_b.tensor, offset=0, ap=[[0, 128], [1, D]])
        nc.gpsimd.dma_start(out=bias_bc[:], in_=bias_src)
        for s in range(2):
            for b in range(GB):
                nc.gpsimd.memset(vA[s][b][:, :, 64:65], 1.0)
            for t in range(3):
                nc.gpsimd.memset(vB[s][t][:, :, 64:65], 1.0)

        def a_part(g, i, st):
            """i in 0..3: q/k chunks for fc=i, 2 vA batches, tail at i=1."""
            xg, xkg, q_fm, k_fm = st
            s = g % 2
            if i == 1:
                # packed v-tail: all 8 batches' tokens 128:144 in one matmul
                tp = ps_pj.tile([128, D], F32, tag="pj", name=f"pjt{g}")
                xkt = xkg[:].rearrange("p f (b c) -> p f b c", c=N)
                for kc in range(4):
                    nc.tensor.matmul(
                        tp[:], xkt[:, kc, :, 128:144],
                        kv_w_sb[:, kc, D:2 * D],
                        start=(kc == 0), stop=(kc == 3))
                tpv = tp[:].rearrange("p (h d) -> p h d", h=H)
                for b in range(GB):
                    dst = vB[s][b // 3][32 * (b % 3):32 * (b % 3) + 16, :,
                                        0:64]
                    if b % 2 == 0:
                        nc.scalar.copy(dst, tpv[16 * b:16 * b + 16])
                    else:
                        nc.vector.tensor_copy(dst, tpv[16 * b:16 * b + 16])
            fc = i
            for isq in range(2):
                src = xg if isq == 0 else xkg
                w_sb = q_w_sb if isq == 0 else kv_w_sb
                dst = q_fm if isq == 0 else k_fm
                for ch in range(NCH):
                    p = ps_pj.tile([128, QKCH], F32, tag="pj",
                                   name=f"pj{g}_{i}_{isq}{ch}")
                    for kc in range(4):
                        nc.tensor.matmul(
                            p[:],
                            w_sb[:, kc, 128 * fc:128 * fc + 128],
                            src[:, kc, QKCH * ch:QKCH * ch + QKCH],
                            start=(kc == 0), stop=(kc == 3))
                    cdst = dst[:, fc, QKCH * ch:QKCH * ch + QKCH]
                    if i == 1 or (ch + isq) % 2 == 0:
                        nc.vector.tensor_copy(cdst, p[:])
                    else:
                        nc.scalar.copy(cdst, p[:])
            for b in (2 * i, 2 * i + 1):
                off = N * b
                p = ps_pj.tile([128, D], F32, tag="pj", name=f"pjv{g}_{b}")
                for kc in range(4):
                    nc.tensor.matmul(
                        p[:], xkg[:, kc, off:off + 128],
                        kv_w_sb[:, kc, D:2 * D],
                        start=(kc == 0), stop=(kc == 3))
                nc.scalar.copy(vA[s][b][:, :, 0:64],
                               p[:].rearrange("p (h d) -> p h d", h=H))

        def b_start(g):
            au = aupool.tile([65, H, GTOK], BF16, tag="au", name=f"au{g}")
            sums = spool.tile([8, GTOK], BF16, tag="sums", name=f"sm{g}")
            recip_bf = spool.tile([8, GTOK], BF16, tag="recipb",
                                  name=f"rb{g}")
            attn_c = acpool.tile([128, 4, GTOK], BF16, tag="ac", name=f"ac{g}")
            return (au, sums, recip_bf, attn_c)

        def b_sc(g, pr, b, q_fm, k_fm):
            """scoresT kA matmuls + exp for one batch."""
            off = N * b
            sc = ps_ms.tile([128, 2, N], F32, tag="ms", name=f"sc{g}_{pr}{b}")
            for j in range(2):
                r0 = 64 * j
                nc.tensor.matmul(
                    sc[:, j, :],
                    k_fm[r0:r0 + 64, pr, off:off + 128],
                    q_fm[r0:r0 + 64, pr, off:off + N],
                    start=True, stop=True)
            e1A = eApool.tile([128, 2, N], BF16, tag="eA", name=f"eA{g}_{b}")
            nc.scalar.activation(e1A[:], sc[:], AF.Exp)
            return e1A

        def b_av(g, pr, b, e1A, e1B, au):
            s = g % 2
            t, pb = b // 3, 32 * (b % 3)
            off = N * b
            av = ps_ms.tile([128, 2, 256], F32, tag="ms",
                            name=f"av{g}_{b}_{pr}")
            for j in range(2):
                h = 2 * pr + j
                nc.tensor.matmul(av[0:65, j, 0:N], vA[s][b][:, h, :],
                                 e1A[:, j, :], start=True, stop=False)
                nc.tensor.matmul(av[0:65, j, 0:N],
                                 vB[s][t][pb:pb + 16, h, :],
                                 e1B[t][pb:pb + 16, j, :],
                                 start=False, stop=True)
            dst = au[:, 2 * pr:2 * pr + 2, off:off + N]
            nc.vector.tensor_copy(dst, av[0:65, :, 0:N])

        def b_part(g, i, st, bst, stash):
            """i in 2..9: idx = i-2 -> (pr, half); 4 batches per part.

            Score matmuls run one batch ahead of av so exp latency hides
            behind the next batch's PE work.
            """
            _, _, q_fm, k_fm = st
            au, sums, recip_bf, attn_c = bst
            idx = i - 2
            pr, half = idx // 2, idx % 2
            if half == 0:
                # packed k-tail scores for all 8 batches of this head pair
                tl = [ps_ms.tile([128, 2, N], F32, tag="ms",
                                 name=f"tl{g}_{pr}_{t}") for t in range(3)]
                for b in range(GB):
                    t, pb = b // 3, 32 * (b % 3)
                    off = N * b
                    for j in range(2):
                        r0 = 64 * j
                        nc.tensor.matmul(
                            tl[t][pb:pb + 16, j, :],
                            k_fm[r0:r0 + 64, pr, off + 128:off + 144],
                            q_fm[r0:r0 + 64, pr, off:off + N],
                            start=True, stop=True)
                e1B = [eBpool.tile([128, 2, N], BF16, tag="eB",
                                   name=f"eB{g}_{pr}_{t}") for t in range(3)]
                for t in range(3):
                    nc.scalar.activation(e1B[t][:], tl[t][:], AF.Exp)
                stash["eB"] = e1B
            e1B = stash["eB"]
            pend = stash["pend"]
            for b in range(4 * half, 4 * half + 4):
                e1A = b_sc(g, pr, b, q_fm, k_fm)
                pend.append((b, e1A))
                if len(pend) > 2:
                    pb_, e1A_ = pend.pop(0)
                    b_av(g, pr, pb_, e1A_, e1B, au)
            if half == 1:
                while pend:
                    pb_, e1A_ = pend.pop(0)
                    b_av(g, pr, pb_, e1A_, e1B, au)
                h0 = 2 * pr
                last_g = (g == NG - 1)
                HT = GTOK // 2
                if last_g and pr == 3:
                    # drain-critical: broadcast raw sums (no DMA round
                    # trip), reciprocal per broadcast tile on DVE
                    for cj in range(2):
                        c0 = HT * cj
                        for j in range(2):
                            h = h0 + j
                            bcs = bcpool.tile([64, HT], BF16, tag="bc",
                                              name=f"bs{g}_{h}_{cj}")
                            nc.gpsimd.partition_broadcast(
                                bcs[:], au[64:65, h, c0:c0 + HT])
                            bc = bcpool.tile([64, HT], BF16, tag="bc",
                                             name=f"bc{g}_{h}_{cj}")
                            with nc.allow_low_precision("bf16 softmax recip"):
                                nc.vector.reciprocal(bc[:], bcs[:])
                            nc.vector.tensor_mul(
                                attn_c[64 * (h % 2):64 * (h % 2) + 64,
                                       h // 2, c0:c0 + HT],
                                au[0:64, h, c0:c0 + HT], bc[:])
                    return
                nc.sync.dma_start(sums[h0:h0 + 2, :],
                                  au[64:65, h0:h0 + 2, :])
                if last_g or pr % 2 == 1:
                    # normalize finished head pairs.  bf16 reciprocal:
                    # downstream math is bf16 anyway.  For the last group
                    # normalize per pair to shorten the drain chain.
                    r0 = 2 * pr if last_g else 2 * pr - 2
                    nh = 2 if last_g else 4
                    with nc.allow_low_precision("softmax recip used in bf16"):
                        nc.vector.reciprocal(recip_bf[r0:r0 + nh, :],
                                             sums[r0:r0 + nh, :])
                    for cj in range(2):
                        c0 = HT * cj
                        for h in range(r0, r0 + nh):
                            bc = bcpool.tile([64, HT], BF16, tag="bc",
                                             name=f"bc{g}_{h}_{cj}")
                            nc.gpsimd.partition_broadcast(
                                bc[:], recip_bf[h:h + 1, c0:c0 + HT])
                            nc.vector.tensor_mul(
                                attn_c[64 * (h % 2):64 * (h % 2) + 64,
                                       h // 2, c0:c0 + HT],
                                au[0:64, h, c0:c0 + HT], bc[:])

        # proj tile indices per part i
        proj_map = {3: (0,), 4: (1,), 5: (2,), 6: (3,), 7: (4,),
                    8: (5, 6), 9: (7, 8)}

        def c_part(g, i, bst):
            attn_c = bst[3]
            g0 = g * GTOK
            # drain slot (no A/B work): use the idle 6-buf score/av psum
            # pool so several proj tiles can be in flight while waiting on
            # the last head pair's normalization
            pool = ps_ms if g == NG - 1 else ps_pj
            tag = "ms" if g == NG - 1 else "pj"
            for t in proj_map.get(i, ()):
                p = pool.tile([128, D], F32, tag=tag, name=f"pp{g}_{t}")
                for fc in range(4):
                    nc.tensor.matmul(
                        p[:], attn_c[:, fc, 128 * t:128 * t + 128],
                        p_w_sb[:, fc, :],
                        start=(fc == 0), stop=(fc == 3))
                o_sb = opool.tile([128, D], F32, tag="osb", name=f"o{g}_{t}")
                nc.vector.tensor_add(o_sb[:], p[:], bias_bc[:])
                nc.sync.dma_start(
                    out[g0 + 128 * t:g0 + 128 * t + 128, :], o_sb[:])

        # 2-stage pipeline: slot g: A(g)+B(g) | C(g-1)
        state = {}
        bst = {}
        stash = {}
        state[0] = st0
        for g in range(NG + 1):
            if 0 < g + 1 < NG:
                state[g + 1] = a_load(g + 1)
            if g < NG:
                bst[g] = b_start(g)
                stash[g] = {"eB": None, "pend": []}
            for i in range(10):
                if g < NG and i < 4:
                    a_part(g, i, state[g])
                if 0 <= g - 1 < NG:
                    c_part(g - 1, i, bst[g - 1])
                if g < NG and 2 <= i:
                    b_part(g, i, state[g], bst[g], stash[g])
            state.pop(g, None)
            stash.pop(g, None)
            bst.pop(g - 1, None)

    nc.compile()
    return nc


def _get_nc():
    if "nc" not in _CACHE:
        _CACHE["nc"] = build()
    return _CACHE["nc"]


def _fm_bf16(a):
    """[tok, D] f32 -> [4, 128, tok] feature-major bf16 chunks."""
    t = np.ascontiguousarray(a.reshape(-1, D).T.astype(bfloat16))
    return t.reshape(4, 128, -1)


def kernel(x, topo_all_fea, kv_w, q_w, proj_w, proj_b, is_end):
    x = np.asarray(x, dtype=np.float32)
    topo = np.asarray(topo_all_fea, dtype=np.float32)
    kv_w = np.asarray(kv_w, dtype=np.float32)
    q_w = np.asarray(q_w, dtype=np.float32)
    proj_w = np.asarray(proj_w, dtype=np.float32)
    proj_b = np.asarray(proj_b, dtype=np.float32)
    end = bool(np.asarray(is_end).item()) if not isinstance(is_end, bool) \
        else is_end

    xk = x + topo if end else x

    kv_wT = np.ascontiguousarray(kv_w.T).astype(bfloat16).reshape(4, 128,
                                                                  2 * D)
    q_wT = np.ascontiguousarray(q_w.T * SCALE).astype(bfloat16).reshape(
        4, 128, D)
    p_wT = np.ascontiguousarray(proj_w.T).astype(bfloat16).reshape(4, 128, D)

    nc = _get_nc()
    in_maps = [
        {"xT": _fm_bf16(x[c * BPC:(c + 1) * BPC]),
         "xkT": _fm_bf16(xk[c * BPC:(c + 1) * BPC]),
         "kv_wT": kv_wT, "q_wT": q_wT, "p_wT": p_wT, "p_b": proj_b}
        for c in range(N_CORES)
    ]
    res = run_bass_kernel_spmd(nc, in_maps, core_ids=list(range(N_CORES)))
    outs = [res.results[c]["out"].reshape(BPC, N, D) for c in range(N_CORES)]
    return np.concatenate(outs, axis=0)


# revision 35
# speedup vs baseline: 1.0415x; 1.0138x over previous
"""Trainium2 Bass kernel for nn_Attention_61443802137307.

Multi-head attention block:
    x_topo = x + topo_all_fea (if is_end)
    kv = x_topo @ kv_w.T ; q = x @ q_w.T (scale hd^-0.5 folded into q_w)
    attn = softmax(q k^T); out = (attn @ v) @ proj_w.T + proj_b

Sharding: data-parallel over batch (dim 0), 32 batches per core x 8 cores.

Per-core design (bf16 activations/weights, feature-major):
  - groups of 8 batches (1152 tokens = 9 full 128-token tiles)
  - q/k projections feature-major into 384-col psum chunks -> bf16 q_fm/k_fm
    (fc-major order so head pair pr is ready after part pr)
  - v token-major per batch ([128,512] psum) + one packed strided matmul
    for all 8 batches' 16-token tails; ones column for softmax sums
  - scoresT per (batch, head-pair): kA [128,2,144] psum; k-tails for 8
    batches packed into three [128,2,144] psum tiles at bases 0/32/64
  - exp on ACT; av contracts tokens, psum row 64 = softmax denominator
  - denominators: per-pr DMA gather, per-half-group DVE reciprocal,
    gpsimd partition_broadcast (Pool), bf16 DVE multiply -> attn_c
  - proj: 9 full [128,512] tiles per group, bias add split ACT/DVE
  - 2-stage pipeline: slot g runs A(g)+B(g) interleaved (B(pr) starts as
    soon as its q/k chunks + vA tiles exist) | C(g-1) = proj of previous
    group. Within B, score matmuls run one batch ahead of av (exp hiding).
"""
import numpy as np
from ml_dtypes import bfloat16

import concourse.bass as bass
import concourse.tile as tile
import concourse.mybir as mybir
from concourse import bacc
from concourse.bass_utils import run_bass_kernel_spmd
from contextlib import ExitStack

F32 = mybir.dt.float32
BF16 = mybir.dt.bfloat16
AF = mybir.ActivationFunctionType

B, N, D = 256, 144, 512
H, HD = 8, 64
SCALE = HD ** -0.5
N_CORES = 8
BPC = B // N_CORES          # 32 batches per core
TOK = BPC * N               # 4608 tokens per core
GB = 8                      # batches per group
NG = BPC // GB              # 4 groups
GTOK = GB * N               # 1152 tokens per group (= 9 * 128)
NTT = GTOK // 128           # 9 full token tiles per group
QKCH = 384                  # q/k projection psum chunk
NCH = GTOK // QKCH          # 3 chunks

_CACHE = {}


def build():
    nc = bacc.Bacc("TRN2", target_bir_lowering=False, debug=False,
                   num_devices=N_CORES)

    xT = nc.dram_tensor("xT", [4, 128, TOK], BF16, kind="ExternalInput").ap()
    xkT = nc.dram_tensor("xkT", [4, 128, TOK], BF16, kind="ExternalInput").ap()
    kv_wT = nc.dram_tensor("kv_wT", [4, 128, 2 * D], BF16,
                           kind="ExternalInput").ap()
    q_wT = nc.dram_tensor("q_wT", [4, 128, D], BF16, kind="ExternalInput").ap()
    p_wT = nc.dram_tensor("p_wT", [4, 128, D], BF16, kind="ExternalInput").ap()
    p_b = nc.dram_tensor("p_b", [D], F32, kind="ExternalInput").ap()
    out = nc.dram_tensor("out", [TOK, D], F32, kind="ExternalOutput").ap()

    with tile.TileContext(nc) as tc, ExitStack() as ctx:
        singles = ctx.enter_context(tc.tile_pool(name="singles", bufs=1))
        xpool = ctx.enter_context(tc.tile_pool(name="xpool", bufs=2))
        qkpool = ctx.enter_context(tc.tile_pool(name="qkpool", bufs=2))
        eApool = ctx.enter_context(tc.tile_pool(name="eApool", bufs=3))
        eBpool = ctx.enter_context(tc.tile_pool(name="eBpool", bufs=6))
        aupool = ctx.enter_context(tc.tile_pool(name="aupool", bufs=2))
        spool = ctx.enter_context(tc.tile_pool(name="spool", bufs=2))
        bcpool = ctx.enter_context(tc.tile_pool(name="bcpool", bufs=3))
        acpool = ctx.enter_context(tc.tile_pool(name="acpool", bufs=2))
        opool = ctx.enter_context(tc.tile_pool(name="opool", bufs=6))
        # 8 psum banks: pj 2 (q/k/v chunks + proj) + ms 6 (score/tail/av)
        ps_pj = ctx.enter_context(tc.tile_pool(name="ps_pj", bufs=2,
                                               space="PSUM"))
        ps_ms = ctx.enter_context(tc.tile_pool(name="ps_ms", bufs=6,
                                               space="PSUM"))

        # --- persistent weights ---
        kv_w_sb = singles.tile([128, 4, 2 * D], BF16)
        q_w_sb = singles.tile([128, 4, D], BF16)
        p_w_sb = singles.tile([128, 4, D], BF16)
        bias_bc = singles.tile([128, D], F32)

        # persistent v tiles (2 pipeline slots); ones columns memset once
        vA = [[singles.tile([128, H, 65], BF16, name=f"vA{s}_{b}")
               for b in range(GB)] for s in range(2)]
        vB = [[singles.tile([128, H, 65], BF16, name=f"vB{s}_{t}")
               for t in range(3)] for s in range(2)]

        def alloc_group_tiles(g):
            xg = xpool.tile([128, 4, GTOK], BF16, tag="xg", name=f"xg{g}")
            xkg = xpool.tile([128, 4, GTOK], BF16, tag="xkg", name=f"xkg{g}")
            q_fm = qkpool.tile([128, 4, GTOK], BF16, tag="qfm", name=f"qf{g}")
            k_fm = qkpool.tile([128, 4, GTOK], BF16, tag="kfm", name=f"kf{g}")
            return (xg, xkg, q_fm, k_fm)

        def a_load(g):
            g0 = g * GTOK
            st = alloc_group_tiles(g)
            xg, xkg = st[0], st[1]
            for kc in range(4):
                nc.sync.dma_start(xg[:, kc, :], xT[kc, :, g0:g0 + GTOK])
                nc.sync.dma_start(xkg[:, kc, :], xkT[kc, :, g0:g0 + GTOK])
            return st

        # prologue: interleave weight and group-0 activation DMAs so the
        # first q-projection matmuls start as early as possible
        st0 = alloc_group_tiles(0)
        for kc in range(4):
            nc.sync.dma_start(q_w_sb[:, kc, :], q_wT[kc])
            nc.sync.dma_start(st0[0][:, kc, :], xT[kc, :, 0:GTOK])
        for kc in range(4):
            nc.sync.dma_start(kv_w_sb[:, kc, :], kv_wT[kc])
            nc.sync.dma_start(st0[1][:, kc, :], xkT[kc, :, 0:GTOK])
        for kc in range(4):
            nc.sync.dma_start(p_w_sb[:, kc, :], p_wT[kc])
        bias_src = bass.AP(tensor=p_b.tensor, offset=0, ap=[[0, 128], [1, D]])
        nc.gpsimd.dma_start(out=bias_bc[:], in_=bias_src)
        for s in range(2):
            for b in range(GB):
                nc.gpsimd.memset(vA[s][b][:, :, 64:65], 1.0)
            for t in range(3):
                nc.gpsimd.memset(vB[s][t][:, :, 64:65], 1.0)

        def a_part(g, i, st):
            """i in 0..3: q/k chunks for fc=i, 2 vA batches, tail at i=1."""
            xg, xkg, q_fm, k_fm = st
            s = g % 2
            if i == 1:
                # packed v-tail: all 8 batches' tokens 128:144 in one matmul
                tp = ps_pj.tile([128, D], F32, tag="pj", name=f"pjt{g}")
                xkt = xkg[:].rearrange("p f (b c) -> p f b c", c=N)
                for kc in range(4):
                    nc.tensor.matmul(
                        tp[:], xkt[:, kc, :, 128:144],
                        kv_w_sb[:, kc, D:2 * D],
                        start=(kc == 0), stop=(kc == 3))
                tpv = tp[:].rearrange("p (h d) -> p h d", h=H)
                for b in range(GB):
                    dst = vB[s][b // 3][32 * (b % 3):32 * (b % 3) + 16, :,
                                        0:64]
                    if b % 2 == 0:
                        nc.scalar.copy(dst, tpv[16 * b:16 * b + 16])
                    else:
                        nc.vector.tensor_copy(dst, tpv[16 * b:16 * b + 16])
            fc = i
            for isq in range(2):
                src = xg if isq == 0 else xkg
                w_sb = q_w_sb if isq == 0 else kv_w_sb
                dst = q_fm if isq == 0 else k_fm
                for ch in range(NCH):
                    p = ps_pj.tile([128, QKCH], F32, tag="pj",
                                   name=f"pj{g}_{i}_{isq}{ch}")
                    for kc in range(4):
                        nc.tensor.matmul(
                            p[:],
                            w_sb[:, kc, 128 * fc:128 * fc + 128],
                            src[:, kc, QKCH * ch:QKCH * ch + QKCH],
                            start=(kc == 0), stop=(kc == 3))
                    cdst = dst[:, fc, QKCH * ch:QKCH * ch + QKCH]
                    if i == 1 or (ch + isq) % 2 == 0:
                        nc.vector.tensor_copy(cdst, p[:])
                    else:
                        nc.scalar.copy(cdst, p[:])
            for b in (2 * i, 2 * i + 1):
                off = N * b
                p = ps_pj.tile([128, D], F32, tag="pj", name=f"pjv{g}_{b}")
                for kc in range(4):
                    nc.tensor.matmul(
                        p[:], xkg[:, kc, off:off + 128],
                        kv_w_sb[:, kc, D:2 * D],
                        start=(kc == 0), stop=(kc == 3))
                nc.scalar.copy(vA[s][b][:, :, 0:64],
                               p[:].rearrange("p (h d) -> p h d", h=H))

        def b_start(g):
            au = aupool.tile([65, H, GTOK], BF16, tag="au", name=f"au{g}")
            sums = spool.tile([8, GTOK], BF16, tag="sums", name=f"sm{g}")
            recip_bf = spool.tile([8, GTOK], BF16, tag="recipb",
                                  name=f"rb{g}")
            attn_c = acpool.tile([128, 4, GTOK], BF16, tag="ac", name=f"ac{g}")
            return (au, sums, recip_bf, attn_c)

        def b_sc(g, pr, b, q_fm, k_fm):
            """scoresT kA matmuls + exp for one batch."""
            off = N * b
            sc = ps_ms.tile([128, 2, N], F32, tag="ms", name=f"sc{g}_{pr}{b}")
            for j in range(2):
                r0 = 64 * j
                nc.tensor.matmul(
                    sc[:, j, :],
                    k_fm[r0:r0 + 64, pr, off:off + 128],
                    q_fm[r0:r0 + 64, pr, off:off + N],
                    start=True, stop=True)
            e1A = eApool.tile([128, 2, N], BF16, tag="eA", name=f"eA{g}_{b}")
            nc.scalar.activation(e1A[:], sc[:], AF.Exp)
            return e1A

        def b_av(g, pr, b, e1A, e1B, au):
            s = g % 2
            t, pb = b // 3, 32 * (b % 3)
            off = N * b
            av = ps_ms.tile([128, 2, 256], F32, tag="ms",
                            name=f"av{g}_{b}_{pr}")
            for j in range(2):
                h = 2 * pr + j
                nc.tensor.matmul(av[0:65, j, 0:N], vA[s][b][:, h, :],
                                 e1A[:, j, :], start=True, stop=False)
                nc.tensor.matmul(av[0:65, j, 0:N],
                                 vB[s][t][pb:pb + 16, h, :],
                                 e1B[t][pb:pb + 16, j, :],
                                 start=False, stop=True)
            dst = au[:, 2 * pr:2 * pr + 2, off:off + N]
            nc.vector.tensor_copy(dst, av[0:65, :, 0:N])

        def b_part(g, i, st, bst, stash):
            """i in 2..9: idx = i-2 -> (pr, half); 4 batches per part.

            Score matmuls run one batch ahead of av so exp latency hides
            behind the next batch's PE work.
            """
            _, _, q_fm, k_fm = st
            au, sums, recip_bf, attn_c = bst
            idx = i - 2
            pr, half = idx // 2, idx % 2
            if half == 0:
                # packed k-tail scores for all 8 batches of this head pair
                tl = [ps_ms.tile([128, 2, N], F32, tag="ms",
                                 name=f"tl{g}_{pr}_{t}") for t in range(3)]
                for b in range(GB):
                    t, pb = b // 3, 32 * (b % 3)
                    off = N * b
                    for j in range(2):
                        r0 = 64 * j
                        nc.tensor.matmul(
                            tl[t][pb:pb + 16, j, :],
                            k_fm[r0:r0 + 64, pr, off + 128:off + 144],
                            q_fm[r0:r0 + 64, pr, off:off + N],
                            start=True, stop=True)
                e1B = [eBpool.tile([128, 2, N], BF16, tag="eB",
                                   name=f"eB{g}_{pr}_{t}") for t in range(3)]
                for t in range(3):
                    nc.scalar.activation(e1B[t][:], tl[t][:], AF.Exp)
                stash["eB"] = e1B
            e1B = stash["eB"]
            pend = stash["pend"]
            for b in range(4 * half, 4 * half + 4):
                e1A = b_sc(g, pr, b, q_fm, k_fm)
                pend.append((b, e1A))
                if len(pend) > 2:
                    pb_, e1A_ = pend.pop(0)
                    b_av(g, pr, pb_, e1A_, e1B, au)
            if half == 1:
                while pend:
                    pb_, e1A_ = pend.pop(0)
                    b_av(g, pr, pb_, e1A_, e1B, au)
                h0 = 2 * pr
                last_g = (g == NG - 1)
                HT = GTOK // 2
                if last_g and pr == 3:
                    # drain-critical: broadcast raw sums (no DMA round
                    # trip), reciprocal per broadcast tile on DVE
                    for cj in range(2):
                        c0 = HT * cj
                        for j in range(2):
                            h = h0 + j
                            bcs = bcpool.tile([64, HT], BF16, tag="bc",
                                              name=f"bs{g}_{h}_{cj}")
                            nc.gpsimd.partition_broadcast(
                                bcs[:], au[64:65, h, c0:c0 + HT])
                            bc = bcpool.tile([64, HT], BF16, tag="bc",
                                             name=f"bc{g}_{h}_{cj}")
                            with nc.allow_low_precision("bf16 softmax recip"):
                                nc.vector.reciprocal(bc[:], bcs[:])
                            nc.vector.tensor_mul(
                                attn_c[64 * (h % 2):64 * (h % 2) + 64,
                                       h // 2, c0:c0 + HT],
                                au[0:64, h, c0:c0 + HT], bc[:])
                    return
                nc.sync.dma_start(sums[h0:h0 + 2, :],
                                  au[64:65, h0:h0 + 2, :])
                if last_g or pr % 2 == 1:
                    # normalize finished head pairs.  bf16 reciprocal:
                    # downstream math is bf16 anyway.  For the last group
                    # normalize per pair to shorten the drain chain.
                    r0 = 2 * pr if last_g else 2 * pr - 2
                    nh = 2 if last_g else 4
                    with nc.allow_low_precision("softmax recip used in bf16"):
                        nc.vector.reciprocal(recip_bf[r0:r0 + nh, :],
                                             sums[r0:r0 + nh, :])
                    for cj in range(2):
                        c0 = HT * cj
                        for h in range(r0, r0 + nh):
                            bc = bcpool.tile([64, HT], BF16, tag="bc",
                                             name=f"bc{g}_{h}_{cj}")
                            nc.gpsimd.partition_broadcast(
                                bc[:], recip_bf[h:h + 1, c0:c0 + HT])
                            nc.vector.tensor_mul(
                                attn_c[64 * (h % 2):64 * (h % 2) + 64,
                                       h // 2, c0:c0 + HT],
                                au[0:64, h, c0:c0 + HT], bc[:])

        # proj tile indices per part i
        proj_map = {3: (0,), 4: (1,), 5: (2,), 6: (3,), 7: (4,),
                    8: (5, 6), 9: (7, 8)}

        def c_part(g, i, bst):
            attn_c = bst[3]
            g0 = g * GTOK
            # drain slot (no A/B work): use the idle 6-buf score/av psum
            # pool so several proj tiles can be in flight while waiting on
            # the last head pair's normalization
            pool = ps_ms if g == NG - 1 else ps_pj
            tag = "ms" if g == NG - 1 else "pj"
            for t in proj_map.get(i, ()):
                p = pool.tile([128, D], F32, tag=tag, name=f"pp{g}_{t}")
                for fc in range(4):
                    nc.tensor.matmul(
                        p[:], attn_c[:, fc, 128 * t:128 * t + 128],
                        p_w_sb[:, fc, :],
                        start=(fc == 0), stop=(fc == 3))
                o_sb = opool.tile([128, D], F32, tag="osb", name=f"o{g}_{t}")
                nc.vector.tensor_add(o_sb[:], p[:], bias_bc[:])
                # drain slot: spread output DMAs over a second HWDGE queue
                eng = nc.scalar if (g == NG - 1 and t % 2 == 1) else nc.sync
                eng.dma_start(
                    out[g0 + 128 * t:g0 + 128 * t + 128, :], o_sb[:])

        # 2-stage pipeline: slot g: A(g)+B(g) | C(g-1)
        state = {}
        bst = {}
        stash = {}
        state[0] = st0
        for g in range(NG + 1):
            if 0 < g + 1 < NG:
                state[g + 1] = a_load(g + 1)
            if g < NG:
                bst[g] = b_start(g)
                stash[g] = {"eB": None, "pend": []}
            for i in range(10):
                if g < NG and i < 4:
                    a_part(g, i, state[g])
                if 0 <= g - 1 < NG:
                    c_part(g - 1, i, bst[g - 1])
                if g < NG and 2 <= i:
                    b_part(g, i, state[g], bst[g], stash[g])
            state.pop(g, None)
            stash.pop(g, None)
            bst.pop(g - 1, None)

    nc.compile()
    return nc


def _get_nc():
    if "nc" not in _CACHE:
        _CACHE["nc"] = build()
    return _CACHE["nc"]


def _fm_bf16(a):
    """[tok, D] f32 -> [4, 128, tok] feature-major bf16 chunks."""
    t = np.ascontiguousarray(a.reshape(-1, D).T.astype(bfloat16))
    return t.reshape(4, 128, -1)


def kernel(x, topo_all_fea, kv_w, q_w, proj_w, proj_b, is_end):
    x = np.asarray(x, dtype=np.float32)
    topo = np.asarray(topo_all_fea, dtype=np.float32)
    kv_w = np.asarray(kv_w, dtype=np.float32)
    q_w = np.asarray(q_w, dtype=np.float32)
    proj_w = np.asarray(proj_w, dtype=np.float32)
    proj_b = np.asarray(proj_b, dtype=np.float32)
    end = bool(np.asarray(is_end).item()) if not isinstance(is_end, bool) \
        else is_end

    xk = x + topo if end else x

    kv_wT = np.ascontiguousarray(kv_w.T).astype(bfloat16).reshape(4, 128,
                                                                  2 * D)
    q_wT = np.ascontiguousarray(q_w.T * SCALE).astype(bfloat16).reshape(
        4, 128, D)
    p_wT = np.ascontiguousarray(proj_w.T).astype(bfloat16).reshape(4, 128, D)

    nc = _get_nc()
    in_maps = [
        {"xT": _fm_bf16(x[c * BPC:(c + 1) * BPC]),
         "xkT": _fm_bf16(xk[c * BPC:(c + 1) * BPC]),
         "kv_wT": kv_wT, "q_wT": q_wT, "p_wT": p_wT, "p_b": proj_b}
        for c in range(N_CORES)
    ]
    res = run_bass_kernel_spmd(nc, in_maps, core_ids=list(range(N_CORES)))
    outs = [res.results[c]["out"].reshape(BPC, N, D) for c in range(N_CORES)]
    return np.concatenate(outs, axis=0)


# revision 36
# speedup vs baseline: 1.0834x; 1.0402x over previous
"""Trainium2 Bass kernel for nn_Attention_61443802137307.

Multi-head attention block:
    x_topo = x + topo_all_fea (if is_end)
    kv = x_topo @ kv_w.T ; q = x @ q_w.T (scale hd^-0.5 folded into q_w)
    attn = softmax(q k^T); out = (attn @ v) @ proj_w.T + proj_b

Sharding: data-parallel over batch (dim 0), 32 batches per core x 8 cores.

Per-core design (bf16 activations/weights, feature-major):
  - groups of 8 batches (1152 tokens = 9 full 128-token tiles)
  - q/k projections feature-major into 384-col psum chunks -> bf16 q_fm/k_fm
    (fc-major order so head pair pr is ready after part pr)
  - v token-major per batch ([128,512] psum) + one packed strided matmul
    for all 8 batches' 16-token tails; ones column for softmax sums
  - scoresT per (batch, head-pair): kA [128,2,144] psum; k-tails for 8
    batches packed into three [128,2,144] psum tiles at bases 0/32/64
  - exp on ACT; av contracts tokens, psum row 64 = softmax denominator
  - denominators: per-pr DMA gather, per-half-group DVE reciprocal,
    gpsimd partition_broadcast (Pool), bf16 DVE multiply -> attn_c
  - proj: 9 full [128,512] tiles per group, bias add split ACT/DVE
  - 2-stage pipeline: slot g runs A(g)+B(g) interleaved (B(pr) starts as
    soon as its q/k chunks + vA tiles exist) | C(g-1) = proj of previous
    group. Within B, score matmuls run one batch ahead of av (exp hiding).
"""
import numpy as np
from ml_dtypes import bfloat16

import concourse.bass as bass
import concourse.tile as tile
import concourse.mybir as mybir
from concourse import bacc
from concourse.bass_utils import run_bass_kernel_spmd
from contextlib import ExitStack

F32 = mybir.dt.float32
BF16 = mybir.dt.bfloat16
AF = mybir.ActivationFunctionType

B, N, D = 256, 144, 512
H, HD = 8, 64
SCALE = HD ** -0.5
N_CORES = 8
BPC = B // N_CORES          # 32 batches per core
TOK = BPC * N               # 4608 tokens per core
GB = 8                      # batches per group
NG = BPC // GB              # 4 groups
GTOK = GB * N               # 1152 tokens per group (= 9 * 128)
NTT = GTOK // 128           # 9 full token tiles per group
QKCH = 384                  # q/k projection psum chunk
NCH = GTOK // QKCH          # 3 chunks

_CACHE = {}


def build():
    nc = bacc.Bacc("TRN2", target_bir_lowering=False, debug=False,
                   num_devices=N_CORES)

    xT = nc.dram_tensor("xT", [4, 128, TOK], BF16, kind="ExternalInput").ap()
    xkT = nc.dram_tensor("xkT", [4, 128, TOK], BF16, kind="ExternalInput").ap()
    kv_wT = nc.dram_tensor("kv_wT", [4, 128, 2 * D], BF16,
                           kind="ExternalInput").ap()
    q_wT = nc.dram_tensor("q_wT", [4, 128, D], BF16, kind="ExternalInput").ap()
    p_wT = nc.dram_tensor("p_wT", [4, 128, D], BF16, kind="ExternalInput").ap()
    p_b = nc.dram_tensor("p_b", [D], F32, kind="ExternalInput").ap()
    out = nc.dram_tensor("out", [TOK, D], F32, kind="ExternalOutput").ap()

    with tile.TileContext(nc) as tc, ExitStack() as ctx:
        singles = ctx.enter_context(tc.tile_pool(name="singles", bufs=1))
        xpool = ctx.enter_context(tc.tile_pool(name="xpool", bufs=2))
        qkpool = ctx.enter_context(tc.tile_pool(name="qkpool", bufs=2))
        eApool = ctx.enter_context(tc.tile_pool(name="eApool", bufs=3))
        eBpool = ctx.enter_context(tc.tile_pool(name="eBpool", bufs=6))
        aupool = ctx.enter_context(tc.tile_pool(name="aupool", bufs=2))
        spool = ctx.enter_context(tc.tile_pool(name="spool", bufs=2))
        bcpool = ctx.enter_context(tc.tile_pool(name="bcpool", bufs=3))
        acpool = ctx.enter_context(tc.tile_pool(name="acpool", bufs=2))
        opool = ctx.enter_context(tc.tile_pool(name="opool", bufs=6))
        # 8 psum banks: pj 2 (q/k/v chunks + proj) + ms 6 (score/tail/av)
        ps_pj = ctx.enter_context(tc.tile_pool(name="ps_pj", bufs=3,
                                               space="PSUM"))
        ps_ms = ctx.enter_context(tc.tile_pool(name="ps_ms", bufs=5,
                                               space="PSUM"))

        # --- persistent weights ---
        kv_w_sb = singles.tile([128, 4, 2 * D], BF16)
        q_w_sb = singles.tile([128, 4, D], BF16)
        p_w_sb = singles.tile([128, 4, D], BF16)
        bias_bc = singles.tile([128, D], F32)

        # persistent v tiles (2 pipeline slots); ones columns memset once
        vA = [[singles.tile([128, H, 65], BF16, name=f"vA{s}_{b}")
               for b in range(GB)] for s in range(2)]
        vB = [[singles.tile([128, H, 65], BF16, name=f"vB{s}_{t}")
               for t in range(3)] for s in range(2)]

        def alloc_group_tiles(g):
            xg = xpool.tile([128, 4, GTOK], BF16, tag="xg", name=f"xg{g}")
            xkg = xpool.tile([128, 4, GTOK], BF16, tag="xkg", name=f"xkg{g}")
            q_fm = qkpool.tile([128, 4, GTOK], BF16, tag="qfm", name=f"qf{g}")
            k_fm = qkpool.tile([128, 4, GTOK], BF16, tag="kfm", name=f"kf{g}")
            return (xg, xkg, q_fm, k_fm)

        def a_load(g):
            g0 = g * GTOK
            st = alloc_group_tiles(g)
            xg, xkg = st[0], st[1]
            for kc in range(4):
                nc.sync.dma_start(xg[:, kc, :], xT[kc, :, g0:g0 + GTOK])
                nc.sync.dma_start(xkg[:, kc, :], xkT[kc, :, g0:g0 + GTOK])
            return st

        # prologue: interleave weight and group-0 activation DMAs so the
        # first q-projection matmuls start as early as possible
        st0 = alloc_group_tiles(0)
        for kc in range(4):
            nc.sync.dma_start(q_w_sb[:, kc, :], q_wT[kc])
            nc.sync.dma_start(st0[0][:, kc, :], xT[kc, :, 0:GTOK])
        for kc in range(4):
            nc.sync.dma_start(kv_w_sb[:, kc, :], kv_wT[kc])
            nc.sync.dma_start(st0[1][:, kc, :], xkT[kc, :, 0:GTOK])
        for kc in range(4):
            nc.sync.dma_start(p_w_sb[:, kc, :], p_wT[kc])
        bias_src = bass.AP(tensor=p_b.tensor, offset=0, ap=[[0, 128], [1, D]])
        nc.gpsimd.dma_start(out=bias_bc[:], in_=bias_src)
        for s in range(2):
            for b in range(GB):
                nc.gpsimd.memset(vA[s][b][:, :, 64:65], 1.0)
            for t in range(3):
                nc.gpsimd.memset(vB[s][t][:, :, 64:65], 1.0)

        def a_part(g, i, st):
            """i in 0..3: q/k chunks for fc=i, 2 vA batches, tail at i=1."""
            xg, xkg, q_fm, k_fm = st
            s = g % 2
            if i == 1:
                # packed v-tail: all 8 batches' tokens 128:144 in one matmul
                tp = ps_pj.tile([128, D], F32, tag="pj", name=f"pjt{g}")
                xkt = xkg[:].rearrange("p f (b c) -> p f b c", c=N)
                for kc in range(4):
                    nc.tensor.matmul(
                        tp[:], xkt[:, kc, :, 128:144],
                        kv_w_sb[:, kc, D:2 * D],
                        start=(kc == 0), stop=(kc == 3))
                tpv = tp[:].rearrange("p (h d) -> p h d", h=H)
                for b in range(GB):
                    dst = vB[s][b // 3][32 * (b % 3):32 * (b % 3) + 16, :,
                                        0:64]
                    if b % 2 == 0:
                        nc.scalar.copy(dst, tpv[16 * b:16 * b + 16])
                    else:
                        nc.vector.tensor_copy(dst, tpv[16 * b:16 * b + 16])
            fc = i
            for isq in range(2):
                src = xg if isq == 0 else xkg
                w_sb = q_w_sb if isq == 0 else kv_w_sb
                dst = q_fm if isq == 0 else k_fm
                for ch in range(NCH):
                    p = ps_pj.tile([128, QKCH], F32, tag="pj",
                                   name=f"pj{g}_{i}_{isq}{ch}")
                    for kc in range(4):
                        nc.tensor.matmul(
                            p[:],
                            w_sb[:, kc, 128 * fc:128 * fc + 128],
                            src[:, kc, QKCH * ch:QKCH * ch + QKCH],
                            start=(kc == 0), stop=(kc == 3))
                    cdst = dst[:, fc, QKCH * ch:QKCH * ch + QKCH]
                    if i == 1 or (ch + isq) % 2 == 0:
                        nc.vector.tensor_copy(cdst, p[:])
                    else:
                        nc.scalar.copy(cdst, p[:])
            for b in (2 * i, 2 * i + 1):
                off = N * b
                p = ps_pj.tile([128, D], F32, tag="pj", name=f"pjv{g}_{b}")
                for kc in range(4):
                    nc.tensor.matmul(
                        p[:], xkg[:, kc, off:off + 128],
                        kv_w_sb[:, kc, D:2 * D],
                        start=(kc == 0), stop=(kc == 3))
                nc.scalar.copy(vA[s][b][:, :, 0:64],
                               p[:].rearrange("p (h d) -> p h d", h=H))

        def b_start(g):
            au = aupool.tile([65, H, GTOK], BF16, tag="au", name=f"au{g}")
            sums = spool.tile([8, GTOK], BF16, tag="sums", name=f"sm{g}")
            recip_bf = spool.tile([8, GTOK], BF16, tag="recipb",
                                  name=f"rb{g}")
            attn_c = acpool.tile([128, 4, GTOK], BF16, tag="ac", name=f"ac{g}")
            return (au, sums, recip_bf, attn_c)

        def b_sc(g, pr, b, q_fm, k_fm):
            """scoresT kA matmuls + exp for one batch."""
            off = N * b
            sc = ps_ms.tile([128, 2, N], F32, tag="ms", name=f"sc{g}_{pr}{b}")
            for j in range(2):
                r0 = 64 * j
                nc.tensor.matmul(
                    sc[:, j, :],
                    k_fm[r0:r0 + 64, pr, off:off + 128],
                    q_fm[r0:r0 + 64, pr, off:off + N],
                    start=True, stop=True)
            e1A = eApool.tile([128, 2, N], BF16, tag="eA", name=f"eA{g}_{b}")
            nc.scalar.activation(e1A[:], sc[:], AF.Exp)
            return e1A

        def b_av(g, pr, b, e1A, e1B, au):
            s = g % 2
            t, pb = b // 3, 32 * (b % 3)
            off = N * b
            av = ps_ms.tile([128, 2, 256], F32, tag="ms",
                            name=f"av{g}_{b}_{pr}")
            for j in range(2):
                h = 2 * pr + j
                nc.tensor.matmul(av[0:65, j, 0:N], vA[s][b][:, h, :],
                                 e1A[:, j, :], start=True, stop=False)
                nc.tensor.matmul(av[0:65, j, 0:N],
                                 vB[s][t][pb:pb + 16, h, :],
                                 e1B[t][pb:pb + 16, j, :],
                                 start=False, stop=True)
            dst = au[:, 2 * pr:2 * pr + 2, off:off + N]
            nc.vector.tensor_copy(dst, av[0:65, :, 0:N])

        def b_part(g, i, st, bst, stash):
            """i in 2..9: idx = i-2 -> (pr, half); 4 batches per part.

            Score matmuls run one batch ahead of av so exp latency hides
            behind the next batch's PE work.
            """
            _, _, q_fm, k_fm = st
            au, sums, recip_bf, attn_c = bst
            idx = i - 2
            pr, half = idx // 2, idx % 2
            if half == 0:
                # packed k-tail scores for all 8 batches of this head pair
                tl = [ps_ms.tile([128, 2, N], F32, tag="ms",
                                 name=f"tl{g}_{pr}_{t}") for t in range(3)]
                for b in range(GB):
                    t, pb = b // 3, 32 * (b % 3)
                    off = N * b
                    for j in range(2):
                        r0 = 64 * j
                        nc.tensor.matmul(
                            tl[t][pb:pb + 16, j, :],
                            k_fm[r0:r0 + 64, pr, off + 128:off + 144],
                            q_fm[r0:r0 + 64, pr, off:off + N],
                            start=True, stop=True)
                e1B = [eBpool.tile([128, 2, N], BF16, tag="eB",
                                   name=f"eB{g}_{pr}_{t}") for t in range(3)]
                for t in range(3):
                    nc.scalar.activation(e1B[t][:], tl[t][:], AF.Exp)
                stash["eB"] = e1B
            e1B = stash["eB"]
            pend = stash["pend"]
            for b in range(4 * half, 4 * half + 4):
                e1A = b_sc(g, pr, b, q_fm, k_fm)
                pend.append((b, e1A))
                if len(pend) > 2:
                    pb_, e1A_ = pend.pop(0)
                    b_av(g, pr, pb_, e1A_, e1B, au)
            if half == 1:
                while pend:
                    pb_, e1A_ = pend.pop(0)
                    b_av(g, pr, pb_, e1A_, e1B, au)
                h0 = 2 * pr
                last_g = (g == NG - 1)
                HT = GTOK // 2
                if last_g and pr == 3:
                    # drain-critical: broadcast raw sums (no DMA round
                    # trip), reciprocal per broadcast tile on DVE
                    for cj in range(2):
                        c0 = HT * cj
                        for j in range(2):
                            h = h0 + j
                            bcs = bcpool.tile([64, HT], BF16, tag="bc",
                                              name=f"bs{g}_{h}_{cj}")
                            nc.gpsimd.partition_broadcast(
                                bcs[:], au[64:65, h, c0:c0 + HT])
                            bc = bcpool.tile([64, HT], BF16, tag="bc",
                                             name=f"bc{g}_{h}_{cj}")
                            with nc.allow_low_precision("bf16 softmax recip"):
                                nc.vector.reciprocal(bc[:], bcs[:])
                            nc.vector.tensor_mul(
                                attn_c[64 * (h % 2):64 * (h % 2) + 64,
                                       h // 2, c0:c0 + HT],
                                au[0:64, h, c0:c0 + HT], bc[:])
                    return
                nc.sync.dma_start(sums[h0:h0 + 2, :],
                                  au[64:65, h0:h0 + 2, :])
                if last_g or pr % 2 == 1:
                    # normalize finished head pairs.  bf16 reciprocal:
                    # downstream math is bf16 anyway.  For the last group
                    # normalize per pair to shorten the drain chain.
                    r0 = 2 * pr if last_g else 2 * pr - 2
                    nh = 2 if last_g else 4
                    with nc.allow_low_precision("softmax recip used in bf16"):
                        nc.vector.reciprocal(recip_bf[r0:r0 + nh, :],
                                             sums[r0:r0 + nh, :])
                    for cj in range(2):
                        c0 = HT * cj
                        for h in range(r0, r0 + nh):
                            bc = bcpool.tile([64, HT], BF16, tag="bc",
                                             name=f"bc{g}_{h}_{cj}")
                            nc.gpsimd.partition_broadcast(
                                bc[:], recip_bf[h:h + 1, c0:c0 + HT])
                            nc.vector.tensor_mul(
                                attn_c[64 * (h % 2):64 * (h % 2) + 64,
                                       h // 2, c0:c0 + HT],
                                au[0:64, h, c0:c0 + HT], bc[:])

        # proj tile indices per part i
        proj_map = {3: (0,), 4: (1,), 5: (2,), 6: (3,), 7: (4,),
                    8: (5, 6), 9: (7, 8)}

        def c_part(g, i, bst):
            attn_c = bst[3]
            g0 = g * GTOK
            # drain slot (no A/B work): use the idle 6-buf score/av psum
            # pool so several proj tiles can be in flight while waiting on
            # the last head pair's normalization
            pool = ps_ms if g == NG - 1 else ps_pj
            tag = "ms" if g == NG - 1 else "pj"
            for t in proj_map.get(i, ()):
                p = pool.tile([128, D], F32, tag=tag, name=f"pp{g}_{t}")
                for fc in range(4):
                    nc.tensor.matmul(
                        p[:], attn_c[:, fc, 128 * t:128 * t + 128],
                        p_w_sb[:, fc, :],
                        start=(fc == 0), stop=(fc == 3))
                o_sb = opool.tile([128, D], F32, tag="osb", name=f"o{g}_{t}")
                nc.vector.tensor_add(o_sb[:], p[:], bias_bc[:])
                # drain slot: spread output DMAs over a second HWDGE queue
                eng = nc.scalar if (g == NG - 1 and t % 2 == 1) else nc.sync
                eng.dma_start(
                    out[g0 + 128 * t:g0 + 128 * t + 128, :], o_sb[:])

        # 2-stage pipeline: slot g: A(g)+B(g) | C(g-1)
        state = {}
        bst = {}
        stash = {}
        state[0] = st0
        for g in range(NG + 1):
            if 0 < g + 1 < NG:
                state[g + 1] = a_load(g + 1)
            if g < NG:
                bst[g] = b_start(g)
                stash[g] = {"eB": None, "pend": []}
            for i in range(10):
                if g < NG and i < 4:
                    a_part(g, i, state[g])
                if 0 <= g - 1 < NG:
                    c_part(g - 1, i, bst[g - 1])
                if g < NG and 2 <= i:
                    b_part(g, i, state[g], bst[g], stash[g])
            state.pop(g, None)
            stash.pop(g, None)
            bst.pop(g - 1, None)

    nc.compile()
    return nc


def _get_nc():
    if "nc" not in _CACHE:
        _CACHE["nc"] = build()
    return _CACHE["nc"]


def _fm_bf16(a):
    """[tok, D] f32 -> [4, 128, tok] feature-major bf16 chunks."""
    t = np.ascontiguousarray(a.reshape(-1, D).T.astype(bfloat16))
    return t.reshape(4, 128, -1)


def kernel(x, topo_all_fea, kv_w, q_w, proj_w, proj_b, is_end):
    x = np.asarray(x, dtype=np.float32)
    topo = np.asarray(topo_all_fea, dtype=np.float32)
    kv_w = np.asarray(kv_w, dtype=np.float32)
    q_w = np.asarray(q_w, dtype=np.float32)
    proj_w = np.asarray(proj_w, dtype=np.float32)
    proj_b = np.asarray(proj_b, dtype=np.float32)
    end = bool(np.asarray(is_end).item()) if not isinstance(is_end, bool) \
        else is_end

    xk = x + topo if end else x

    kv_wT = np.ascontiguousarray(kv_w.T).astype(bfloat16).reshape(4, 128,
                                                                  2 * D)
    q_wT = np.ascontiguousarray(q_w.T * SCALE).astype(bfloat16).reshape(
        4, 128, D)
    p_wT = np.ascontiguousarray(proj_w.T).astype(bfloat16).reshape(4, 128, D)

    nc = _get_nc()
    in_maps = [
        {"xT": _fm_bf16(x[c * BPC:(c + 1) * BPC]),
         "xkT": _fm_bf16(xk[c * BPC:(c + 1) * BPC]),
         "kv_wT": kv_wT, "q_wT": q_wT, "p_wT": p_wT, "p_b": proj_b}
        for c in range(N_CORES)
    ]
    res = run_bass_kernel_spmd(nc, in_maps, core_ids=list(range(N_CORES)))
    outs = [res.results[c]["out"].reshape(BPC, N, D) for c in range(N_CORES)]
    return np.concatenate(outs, axis=0)


# revision 37
# speedup vs baseline: 1.1054x; 1.0203x over previous
"""Trainium2 Bass kernel for nn_Attention_61443802137307.

Multi-head attention block:
    x_topo = x + topo_all_fea (if is_end)
    kv = x_topo @ kv_w.T ; q = x @ q_w.T (scale hd^-0.5 folded into q_w)
    attn = softmax(q k^T); out = (attn @ v) @ proj_w.T + proj_b

Sharding: data-parallel over batch (dim 0), 32 batches per core x 8 cores.

Per-core design (bf16 activations/weights, feature-major):
  - groups of 8 batches (1152 tokens = 9 full 128-token tiles)
  - q/k projections feature-major into 384-col psum chunks -> bf16 q_fm/k_fm
    (fc-major order so head pair pr is ready after part pr)
  - v token-major per batch ([128,512] psum) + one packed strided matmul
    for all 8 batches' 16-token tails; ones column for softmax sums
  - scoresT per (batch, head-pair): kA [128,2,144] psum; k-tails for 8
    batches packed into three [128,2,144] psum tiles at bases 0/32/64
  - exp on ACT; av contracts tokens, psum row 64 = softmax denominator
  - denominators: per-pr DMA gather, per-half-group DVE reciprocal,
    gpsimd partition_broadcast (Pool), bf16 DVE multiply -> attn_c
  - proj: 9 full [128,512] tiles per group, bias add split ACT/DVE
  - 2-stage pipeline: slot g runs A(g)+B(g) interleaved (B(pr) starts as
    soon as its q/k chunks + vA tiles exist) | C(g-1) = proj of previous
    group. Within B, score matmuls run one batch ahead of av (exp hiding).
"""
import numpy as np
from ml_dtypes import bfloat16

import concourse.bass as bass
import concourse.tile as tile
import concourse.mybir as mybir
from concourse import bacc
from concourse.bass_utils import run_bass_kernel_spmd
from contextlib import ExitStack

F32 = mybir.dt.float32
BF16 = mybir.dt.bfloat16
AF = mybir.ActivationFunctionType

B, N, D = 256, 144, 512
H, HD = 8, 64
SCALE = HD ** -0.5
N_CORES = 8
BPC = B // N_CORES          # 32 batches per core
TOK = BPC * N               # 4608 tokens per core
GB = 8                      # batches per group
NG = BPC // GB              # 4 groups
GTOK = GB * N               # 1152 tokens per group (= 9 * 128)
NTT = GTOK // 128           # 9 full token tiles per group
QKCH = 384                  # q/k projection psum chunk
NCH = GTOK // QKCH          # 3 chunks

_CACHE = {}


def build():
    nc = bacc.Bacc("TRN2", target_bir_lowering=False, debug=False,
                   num_devices=N_CORES)

    xT = nc.dram_tensor("xT", [4, 128, TOK], BF16, kind="ExternalInput").ap()
    xkT = nc.dram_tensor("xkT", [4, 128, TOK], BF16, kind="ExternalInput").ap()
    kv_wT = nc.dram_tensor("kv_wT", [4, 128, 2 * D], BF16,
                           kind="ExternalInput").ap()
    q_wT = nc.dram_tensor("q_wT", [4, 128, D], BF16, kind="ExternalInput").ap()
    p_wT = nc.dram_tensor("p_wT", [4, 128, D], BF16, kind="ExternalInput").ap()
    p_b = nc.dram_tensor("p_b", [D], F32, kind="ExternalInput").ap()
    out = nc.dram_tensor("out", [TOK, D], F32, kind="ExternalOutput").ap()

    with tile.TileContext(nc) as tc, ExitStack() as ctx:
        singles = ctx.enter_context(tc.tile_pool(name="singles", bufs=1))
        xpool = ctx.enter_context(tc.tile_pool(name="xpool", bufs=2))
        qkpool = ctx.enter_context(tc.tile_pool(name="qkpool", bufs=2))
        eApool = ctx.enter_context(tc.tile_pool(name="eApool", bufs=3))
        eBpool = ctx.enter_context(tc.tile_pool(name="eBpool", bufs=6))
        aupool = ctx.enter_context(tc.tile_pool(name="aupool", bufs=2))
        spool = ctx.enter_context(tc.tile_pool(name="spool", bufs=2))
        bcpool = ctx.enter_context(tc.tile_pool(name="bcpool", bufs=3))
        acpool = ctx.enter_context(tc.tile_pool(name="acpool", bufs=2))
        opool = ctx.enter_context(tc.tile_pool(name="opool", bufs=6))
        # 8 psum banks: pj 2 (q/k/v chunks + proj) + ms 6 (score/tail/av)
        ps_pj = ctx.enter_context(tc.tile_pool(name="ps_pj", bufs=3,
                                               space="PSUM"))
        ps_ms = ctx.enter_context(tc.tile_pool(name="ps_ms", bufs=5,
                                               space="PSUM"))

        # --- persistent weights ---
        kv_w_sb = singles.tile([128, 4, 2 * D], BF16)
        q_w_sb = singles.tile([128, 4, D], BF16)
        p_w_sb = singles.tile([128, 4, D], BF16)
        bias_bc = singles.tile([128, D], F32)

        # persistent v tiles (2 pipeline slots); ones columns memset once
        vA = [[singles.tile([128, H, 65], BF16, name=f"vA{s}_{b}")
               for b in range(GB)] for s in range(2)]
        vB = [[singles.tile([128, H, 65], BF16, name=f"vB{s}_{t}")
               for t in range(3)] for s in range(2)]

        def alloc_group_tiles(g):
            xg = xpool.tile([128, 4, GTOK], BF16, tag="xg", name=f"xg{g}")
            xkg = xpool.tile([128, 4, GTOK], BF16, tag="xkg", name=f"xkg{g}")
            q_fm = qkpool.tile([128, 4, GTOK], BF16, tag="qfm", name=f"qf{g}")
            k_fm = qkpool.tile([128, 4, GTOK], BF16, tag="kfm", name=f"kf{g}")
            return (xg, xkg, q_fm, k_fm)

        def a_load(g):
            g0 = g * GTOK
            st = alloc_group_tiles(g)
            xg, xkg = st[0], st[1]
            for kc in range(4):
                nc.sync.dma_start(xg[:, kc, :], xT[kc, :, g0:g0 + GTOK])
                nc.sync.dma_start(xkg[:, kc, :], xkT[kc, :, g0:g0 + GTOK])
            return st

        # prologue: interleave weight and group-0 activation DMAs so the
        # first q-projection matmuls start as early as possible
        st0 = alloc_group_tiles(0)
        for kc in range(4):
            nc.sync.dma_start(q_w_sb[:, kc, :], q_wT[kc])
            nc.sync.dma_start(st0[0][:, kc, :], xT[kc, :, 0:GTOK])
        for kc in range(4):
            nc.sync.dma_start(kv_w_sb[:, kc, :], kv_wT[kc])
            nc.sync.dma_start(st0[1][:, kc, :], xkT[kc, :, 0:GTOK])
        for kc in range(4):
            nc.sync.dma_start(p_w_sb[:, kc, :], p_wT[kc])
        bias_src = bass.AP(tensor=p_b.tensor, offset=0, ap=[[0, 128], [1, D]])
        nc.gpsimd.dma_start(out=bias_bc[:], in_=bias_src)
        for s in range(2):
            for b in range(GB):
                nc.gpsimd.memset(vA[s][b][:, :, 64:65], 1.0)
            for t in range(3):
                nc.gpsimd.memset(vB[s][t][:, :, 64:65], 1.0)

        def a_part(g, i, st):
            """i in 0..3: q/k chunks for fc=i, 2 vA batches, tail at i=1."""
            xg, xkg, q_fm, k_fm = st
            s = g % 2
            if i == 1:
                # packed v-tail: all 8 batches' tokens 128:144 in one matmul
                tp = ps_pj.tile([128, D], F32, tag="pj", name=f"pjt{g}")
                xkt = xkg[:].rearrange("p f (b c) -> p f b c", c=N)
                for kc in range(4):
                    nc.tensor.matmul(
                        tp[:], xkt[:, kc, :, 128:144],
                        kv_w_sb[:, kc, D:2 * D],
                        start=(kc == 0), stop=(kc == 3))
                tpv = tp[:].rearrange("p (h d) -> p h d", h=H)
                for b in range(GB):
                    dst = vB[s][b // 3][32 * (b % 3):32 * (b % 3) + 16, :,
                                        0:64]
                    if b % 2 == 0:
                        nc.scalar.copy(dst, tpv[16 * b:16 * b + 16])
                    else:
                        nc.vector.tensor_copy(dst, tpv[16 * b:16 * b + 16])
            fc = i
            for isq in range(2):
                src = xg if isq == 0 else xkg
                w_sb = q_w_sb if isq == 0 else kv_w_sb
                dst = q_fm if isq == 0 else k_fm
                for ch in range(NCH):
                    p = ps_pj.tile([128, QKCH], F32, tag="pj",
                                   name=f"pj{g}_{i}_{isq}{ch}")
                    for kc in range(4):
                        nc.tensor.matmul(
                            p[:],
                            w_sb[:, kc, 128 * fc:128 * fc + 128],
                            src[:, kc, QKCH * ch:QKCH * ch + QKCH],
                            start=(kc == 0), stop=(kc == 3))
                    cdst = dst[:, fc, QKCH * ch:QKCH * ch + QKCH]
                    if i == 1 or (ch + isq) % 2 == 0:
                        nc.vector.tensor_copy(cdst, p[:])
                    else:
                        nc.scalar.copy(cdst, p[:])
            for b in (2 * i, 2 * i + 1):
                off = N * b
                p = ps_pj.tile([128, D], F32, tag="pj", name=f"pjv{g}_{b}")
                for kc in range(4):
                    nc.tensor.matmul(
                        p[:], xkg[:, kc, off:off + 128],
                        kv_w_sb[:, kc, D:2 * D],
                        start=(kc == 0), stop=(kc == 3))
                nc.scalar.copy(vA[s][b][:, :, 0:64],
                               p[:].rearrange("p (h d) -> p h d", h=H))

        def b_start(g):
            au = aupool.tile([65, H, GTOK], BF16, tag="au", name=f"au{g}")
            sums = spool.tile([8, GTOK], BF16, tag="sums", name=f"sm{g}")
            recip_bf = spool.tile([8, GTOK], BF16, tag="recipb",
                                  name=f"rb{g}")
            attn_c = acpool.tile([128, 4, GTOK], BF16, tag="ac", name=f"ac{g}")
            return (au, sums, recip_bf, attn_c)

        def b_sc(g, pr, b, q_fm, k_fm):
            """scoresT kA matmuls + exp for one batch."""
            off = N * b
            sc = ps_ms.tile([128, 2, N], F32, tag="ms", name=f"sc{g}_{pr}{b}")
            for j in range(2):
                r0 = 64 * j
                nc.tensor.matmul(
                    sc[:, j, :],
                    k_fm[r0:r0 + 64, pr, off:off + 128],
                    q_fm[r0:r0 + 64, pr, off:off + N],
                    start=True, stop=True)
            e1A = eApool.tile([128, 2, N], BF16, tag="eA", name=f"eA{g}_{b}")
            nc.scalar.activation(e1A[:], sc[:], AF.Exp)
            return e1A

        def b_av(g, pr, b, e1A, e1B, au):
            s = g % 2
            t, pb = b // 3, 32 * (b % 3)
            off = N * b
            av = ps_ms.tile([128, 2, 256], F32, tag="ms",
                            name=f"av{g}_{b}_{pr}")
            for j in range(2):
                h = 2 * pr + j
                nc.tensor.matmul(av[0:65, j, 0:N], vA[s][b][:, h, :],
                                 e1A[:, j, :], start=True, stop=False)
                nc.tensor.matmul(av[0:65, j, 0:N],
                                 vB[s][t][pb:pb + 16, h, :],
                                 e1B[t][pb:pb + 16, j, :],
                                 start=False, stop=True)
            dst = au[:, 2 * pr:2 * pr + 2, off:off + N]
            if b % 4 == 1:
                nc.scalar.copy(dst, av[0:65, :, 0:N])
            else:
                nc.vector.tensor_copy(dst, av[0:65, :, 0:N])

        def b_part(g, i, st, bst, stash):
            """i in 2..9: idx = i-2 -> (pr, half); 4 batches per part.

            Score matmuls run one batch ahead of av so exp latency hides
            behind the next batch's PE work.
            """
            _, _, q_fm, k_fm = st
            au, sums, recip_bf, attn_c = bst
            idx = i - 2
            pr, half = idx // 2, idx % 2
            if half == 0:
                # packed k-tail scores for all 8 batches of this head pair
                tl = [ps_ms.tile([128, 2, N], F32, tag="ms",
                                 name=f"tl{g}_{pr}_{t}") for t in range(3)]
                for b in range(GB):
                    t, pb = b // 3, 32 * (b % 3)
                    off = N * b
                    for j in range(2):
                        r0 = 64 * j
                        nc.tensor.matmul(
                            tl[t][pb:pb + 16, j, :],
                            k_fm[r0:r0 + 64, pr, off + 128:off + 144],
                            q_fm[r0:r0 + 64, pr, off:off + N],
                            start=True, stop=True)
                e1B = [eBpool.tile([128, 2, N], BF16, tag="eB",
                                   name=f"eB{g}_{pr}_{t}") for t in range(3)]
                for t in range(3):
                    nc.scalar.activation(e1B[t][:], tl[t][:], AF.Exp)
                stash["eB"] = e1B
            e1B = stash["eB"]
            pend = stash["pend"]
            for b in range(4 * half, 4 * half + 4):
                e1A = b_sc(g, pr, b, q_fm, k_fm)
                pend.append((b, e1A))
                if len(pend) > 2:
                    pb_, e1A_ = pend.pop(0)
                    b_av(g, pr, pb_, e1A_, e1B, au)
            if half == 1:
                while pend:
                    pb_, e1A_ = pend.pop(0)
                    b_av(g, pr, pb_, e1A_, e1B, au)
                h0 = 2 * pr
                last_g = (g == NG - 1)
                HT = GTOK // 2
                if last_g and pr == 3:
                    # drain-critical: broadcast raw sums (no DMA round
                    # trip), reciprocal per broadcast tile on DVE
                    for cj in range(2):
                        c0 = HT * cj
                        for j in range(2):
                            h = h0 + j
                            bcs = bcpool.tile([64, HT], BF16, tag="bc",
                                              name=f"bs{g}_{h}_{cj}")
                            nc.gpsimd.partition_broadcast(
                                bcs[:], au[64:65, h, c0:c0 + HT])
                            bc = bcpool.tile([64, HT], BF16, tag="bc",
                                             name=f"bc{g}_{h}_{cj}")
                            with nc.allow_low_precision("bf16 softmax recip"):
                                nc.vector.reciprocal(bc[:], bcs[:])
                            nc.vector.tensor_mul(
                                attn_c[64 * (h % 2):64 * (h % 2) + 64,
                                       h // 2, c0:c0 + HT],
                                au[0:64, h, c0:c0 + HT], bc[:])
                    return
                nc.sync.dma_start(sums[h0:h0 + 2, :],
                                  au[64:65, h0:h0 + 2, :])
                if last_g or pr % 2 == 1:
                    # normalize finished head pairs.  bf16 reciprocal:
                    # downstream math is bf16 anyway.  For the last group
                    # normalize per pair to shorten the drain chain.
                    r0 = 2 * pr if last_g else 2 * pr - 2
                    nh = 2 if last_g else 4
                    with nc.allow_low_precision("softmax recip used in bf16"):
                        nc.vector.reciprocal(recip_bf[r0:r0 + nh, :],
                                             sums[r0:r0 + nh, :])
                    for cj in range(2):
                        c0 = HT * cj
                        for h in range(r0, r0 + nh):
                            bc = bcpool.tile([64, HT], BF16, tag="bc",
                                             name=f"bc{g}_{h}_{cj}")
                            nc.gpsimd.partition_broadcast(
                                bc[:], recip_bf[h:h + 1, c0:c0 + HT])
                            nc.vector.tensor_mul(
                                attn_c[64 * (h % 2):64 * (h % 2) + 64,
                                       h // 2, c0:c0 + HT],
                                au[0:64, h, c0:c0 + HT], bc[:])

        # proj tile indices per part i
        proj_map = {3: (0,), 4: (1,), 5: (2,), 6: (3,), 7: (4,),
                    8: (5, 6), 9: (7, 8)}

        def c_part(g, i, bst):
            attn_c = bst[3]
            g0 = g * GTOK
            # drain slot (no A/B work): use the idle 6-buf score/av psum
            # pool so several proj tiles can be in flight while waiting on
            # the last head pair's normalization
            pool = ps_ms if g == NG - 1 else ps_pj
            tag = "ms" if g == NG - 1 else "pj"
            for t in proj_map.get(i, ()):
                p = pool.tile([128, D], F32, tag=tag, name=f"pp{g}_{t}")
                for fc in range(4):
                    nc.tensor.matmul(
                        p[:], attn_c[:, fc, 128 * t:128 * t + 128],
                        p_w_sb[:, fc, :],
                        start=(fc == 0), stop=(fc == 3))
                o_sb = opool.tile([128, D], F32, tag="osb", name=f"o{g}_{t}")
                if g == NG - 1:
                    nc.vector.tensor_add(o_sb[:], p[:], bias_bc[:])
                else:
                    nc.gpsimd.tensor_add(o_sb[:], p[:], bias_bc[:])
                # drain slot: spread output DMAs over a second HWDGE queue
                eng = nc.scalar if (g == NG - 1 and t % 2 == 1) else nc.sync
                eng.dma_start(
                    out[g0 + 128 * t:g0 + 128 * t + 128, :], o_sb[:])

        # 2-stage pipeline: slot g: A(g)+B(g) | C(g-1)
        state = {}
        bst = {}
        stash = {}
        state[0] = st0
        for g in range(NG + 1):
            if 0 < g + 1 < NG:
                state[g + 1] = a_load(g + 1)
            if g < NG:
                bst[g] = b_start(g)
                stash[g] = {"eB": None, "pend": []}
            for i in range(10):
                if g < NG and i < 4:
                    a_part(g, i, state[g])
                if 0 <= g - 1 < NG:
                    c_part(g - 1, i, bst[g - 1])
                if g < NG and 2 <= i:
                    b_part(g, i, state[g], bst[g], stash[g])
            state.pop(g, None)
            stash.pop(g, None)
            bst.pop(g - 1, None)

    nc.compile()
    return nc


def _get_nc():
    if "nc" not in _CACHE:
        _CACHE["nc"] = build()
    return _CACHE["nc"]


def _fm_bf16(a):
    """[tok, D] f32 -> [4, 128, tok] feature-major bf16 chunks."""
    t = np.ascontiguousarray(a.reshape(-1, D).T.astype(bfloat16))
    return t.reshape(4, 128, -1)


def kernel(x, topo_all_fea, kv_w, q_w, proj_w, proj_b, is_end):
    x = np.asarray(x, dtype=np.float32)
    topo = np.asarray(topo_all_fea, dtype=np.float32)
    kv_w = np.asarray(kv_w, dtype=np.float32)
    q_w = np.asarray(q_w, dtype=np.float32)
    proj_w = np.asarray(proj_w, dtype=np.float32)
    proj_b = np.asarray(proj_b, dtype=np.float32)
    end = bool(np.asarray(is_end).item()) if not isinstance(is_end, bool) \
        else is_end

    xk = x + topo if end else x

    kv_wT = np.ascontiguousarray(kv_w.T).astype(bfloat16).reshape(4, 128,
                                                                  2 * D)
    q_wT = np.ascontiguousarray(q_w.T * SCALE).astype(bfloat16).reshape(
        4, 128, D)
    p_wT = np.ascontiguousarray(proj_w.T).astype(bfloat16).reshape(4, 128, D)

    nc = _get_nc()
    in_maps = [
        {"xT": _fm_bf16(x[c * BPC:(c + 1) * BPC]),
         "xkT": _fm_bf16(xk[c * BPC:(c + 1) * BPC]),
         "kv_wT": kv_wT, "q_wT": q_wT, "p_wT": p_wT, "p_b": proj_b}
        for c in range(N_CORES)
    ]
    res = run_bass_kernel_spmd(nc, in_maps, core_ids=list(range(N_CORES)))
    outs = [res.results[c]["out"].reshape(BPC, N, D) for c in range(N_CORES)]
    return np.concatenate(outs, axis=0)
